# revision 42
# baseline (speedup 1.0000x reference)
"""Multi-head attention (B=2, S=2048, D=1024, H=16) on 8 Trainium2 NeuronCores.

Sharding: core c handles batch b = c//4 and head group g = c%4 (4 heads, 256
model dims).  Each core computes q/k/v projections for its heads, attention,
and a partial output projection (row-parallel over its 256 head dims); the
host sums the 4 partials per batch and adds the bias.

Layouts / engine plan (v2 — PV reoriented, softmax normalize folded):
  xT  [d, s] f32r (host pre-transposed); wq/wk/wv f32r; wo bf16.
  qT/kT [e(128 = head pair), s] f32r; scores computed transposed [ks, qs] in
  PSUM, exp on ACT -> p [ks, qs] bf16 in SBUF (ACT does nothing else, so the
  exp stream is the makespan spine at ~133us).
  PV is out[q(128), e(65)] with K=128 (lhsT = p block, rhs = v[ks, e+1] with a
  trailing ones column): the 65-col free dim runs at 1 cyc/row in bf16, and
  column 64 accumulates the softmax denominator per-q (a per-PARTITION
  scalar).  Normalization is folded into the PSUM evacuation: DVE
  reciprocal of the denominator column + tensor_scalar multiply -> ctxQ bf16.
  ctxQ [q, e-pair] -> ctxT [e-pair, q] via the XBAR dma transpose (bf16,
  SBUF->SBUF), then the output projection contracts K=128 per head pair.
  Output partials staged f16 (DVE/Pool copies) and DMA'd out.

PSUM (8 banks): sc0+sc1 [128,1024] f32 (4) + pva/pvb [128,7,65] + pvs
[128,2,65] (3, one pool each so they are bank-aligned) + pj [128,512] (1).

Emission is a 64-step pipeline (4 (m,qh) groups x 16 ks blocks): each step
emits the two score matmuls + exps, then pops PV steps and "filler" units
(qkv projection chunks, out-projection tiles) against a rows-emitted pacing
target so the PE queue tracks the ACT stream without starving it.
"""

import os
import sys

import numpy as np

for _p in ("/opt/trn_rl_repo", "/root/.axon_site/_ro/trn_rl_repo"):
    if os.path.isdir(_p) and _p not in sys.path:
        sys.path.insert(0, _p)

import bass_rust
import concourse.bass as bass
import concourse.mybir as mybir
import concourse.tile as tile
from concourse.bass_utils import run_bass_kernel_spmd
from concourse.vector_clock import ScopedClock, VectorClock
from contextlib import ExitStack
from collections import deque

F32 = mybir.dt.float32
F32R = mybir.dt.float32r
BF16 = mybir.dt.bfloat16
F16 = mybir.dt.float16
EXP = mybir.ActivationFunctionType.Exp

B = 2
S = 2048
D = 1024
H = 16
HD = 64
NCORES = 8
GROUPS = 4          # head groups (cores per batch)
HG = H // GROUPS    # heads per core = 4
E = HG * HD         # head dims per core = 256
KT = D // 128       # contraction tiles over model dim = 8
SB = S // 128       # 128-row s blocks = 16
NP = 36             # p-tile ring size (bf16 [128,1024] tiles)

_carrier_counter = [0]


def _split_multi_waits(ordered):
    """This walrus build allows one sync wait per instruction; Tile's wait
    assignment can attach several.  Hoist extras onto same-engine InstNoOp
    carriers placed immediately before the instruction."""
    for bb_name, insts in ordered.items():
        new_list = []
        for inst in insts:
            si = inst.sync_info
            waits = list(si.on_wait) if si is not None else []
            if len(waits) > 1:
                for w in waits[:-1]:
                    _carrier_counter[0] += 1
                    carrier = mybir.InstNoOp(
                        name=f"I-waitc-{_carrier_counter[0]}", ins=[], outs=[]
                    )
                    carrier.engine = inst.engine
                    carrier.sync_info = bass_rust.SyncInfo(on_wait=[w], on_update=[])
                    new_list.append(carrier)
                inst.sync_info = bass_rust.SyncInfo(
                    on_wait=[waits[-1]],
                    on_update=list(si.on_update) if si is not None else [],
                )
            new_list.append(inst)
        ordered[bb_name] = new_list


class _TileContext(tile.TileContext):
    """TileContext adapted to the one-sync-wait-per-instruction walrus."""

    def _lower_ordered_insts(self, ordered):
        _split_multi_waits(ordered)
        return super()._lower_ordered_insts(ordered)

    def _drain_and_barrier(self, tick_clock, wait_clock):
        gc = tick_clock.global_clock
        for proc in range(len(gc)):
            if gc[proc] <= 0:
                continue
            cur = VectorClock([0 if i == proc else gc[i] for i in range(len(gc))])
            nop = self.nc.sync.nop()
            wait_clock.add_sem_waits(
                nop.ins, ScopedClock({None: gc}), ScopedClock({None: cur})
            )
        drain_inst = self.nc.sync.drain()
        wait_clock.add_sem_waits(
            drain_inst.ins, ScopedClock({None: gc}), ScopedClock({None: gc.copy()})
        )
        self.nc.all_engine_barrier()
        assert self.sems is not None
        popped = self.nc._tile_sem_poison_stack.pop()
        assert popped is self._sem_poison
        self.nc.clear_and_free_semaphores(list(self.sems.allocated().values()))
        self.nc.all_engine_barrier()


def build_nc():
    nc = bass.Bass()
    xT = nc.declare_dram_parameter("xT", [D, S], BF16, isOutput=False)
    wqT = nc.declare_dram_parameter("wqT", [D, E], BF16, isOutput=False)
    wkT = nc.declare_dram_parameter("wkT", [D, E], BF16, isOutput=False)
    wvT = nc.declare_dram_parameter("wvT", [D, E], BF16, isOutput=False)
    woT = nc.declare_dram_parameter("woT", [E, D], BF16, isOutput=False)
    identT = nc.declare_dram_parameter("identT", [128, 128], BF16, isOutput=False)
    out = nc.declare_dram_parameter("out_partial", [S, D], F16, isOutput=True)

    with _TileContext(nc) as tc, ExitStack() as ctx:
        sb = ctx.enter_context(tc.tile_pool(name="sb", bufs=1))
        x_sb = sb.tile([128, KT, S], BF16, tag="x", name="x_sb")
        wq_sb = sb.tile([128, KT, E], BF16, tag="wq", name="wq_sb")
        wk_sb = sb.tile([128, KT, E], BF16, tag="wk", name="wk_sb")
        wv_sb = sb.tile([128, KT, E], BF16, tag="wv", name="wv_sb")
        wo_sb = sb.tile([128, 2, D], BF16, tag="wo", name="wo_sb")
        qT = [sb.tile([128, S], BF16, tag=f"qT{m}", name=f"qT{m}") for m in range(2)]
        kT = [sb.tile([128, S], BF16, tag=f"kT{m}", name=f"kT{m}") for m in range(2)]
        v_sb = sb.tile([128, SB, HG, HD + 1], BF16, tag="v", name="v_sb")
        ctxQ = [sb.tile([128, SB, 128], BF16, tag=f"cq{m}", name=f"cq{m}")
                for m in range(2)]
        ctxT = [sb.tile([128, S], BF16, tag=f"ct{m}", name=f"ct{m}")
                for m in range(2)]
        rec = [sb.tile([128, 2, SB], F32, tag=f"rec{m}", name=f"rec{m}")
               for m in range(2)]
        ident_sb = sb.tile([128, 128], BF16, tag="ident", name="ident_sb")

        p_pool = ctx.enter_context(tc.tile_pool(name="pp", bufs=1))
        st_pool = ctx.enter_context(tc.tile_pool(name="st", bufs=1))

        ps_sc = ctx.enter_context(tc.tile_pool(name="ps_sc", bufs=1, space="PSUM"))
        ps_pva = ctx.enter_context(tc.tile_pool(name="ps_pva", bufs=1, space="PSUM"))
        ps_pvb = ctx.enter_context(tc.tile_pool(name="ps_pvb", bufs=1, space="PSUM"))
        ps_pvs = ctx.enter_context(tc.tile_pool(name="ps_pvs", bufs=1, space="PSUM"))
        ps_pj = ctx.enter_context(tc.tile_pool(name="ps_pj", bufs=1, space="PSUM"))

        sc = [ps_sc.tile([128, 1024], F32, tag=f"sc{r}", name=f"sc{r}")
              for r in range(2)]
        pva = ps_pva.tile([128, 7, HD + 1], F32, tag="pva", name="pva")
        pvb = ps_pvb.tile([128, 7, HD + 1], F32, tag="pvb", name="pvb")
        pvs = ps_pvs.tile([128, 2, HD + 1], F32, tag="pvs", name="pvs")
        pj = ps_pj.tile([128, 512], F32, tag="pj", name="pj")

        # ---- input DMAs ----
        # Priority-interleaved so the phase-A projections start after the
        # first (wq_k, x_k) pair instead of after the whole weight load:
        # sync queue carries the critical path (wq/wk + x for qs 0:1024),
        # gpsimd the rest (wv, wo, x tail).
        # preload the Exp activation table while DMAs stream (saves the
        # 1.3us implicit table load before the first real exp)
        warm = sb.tile([1, 512], BF16, tag="warm", name="warm")
        warmf = sb.tile([1, 8], F32, tag="warmf", name="warmf")
        nc.vector.memset(warm[0:1, :], 0.0)
        nc.vector.memset(warmf[0:1, 0:8], 0.0)
        nc.scalar.activation(warmf[0:1, 0:8], warmf[0:1, 0:8], EXP)
        # PE p-state warmup: a dep-free dummy matmul train ramps the tensor
        # engine to full clock before the first projection lands (~6us in)
        for _ in range(14):
            nc.tensor.matmul(pj[0:1, 0:512], warm[0:1, 0:1],
                             warm[0:1, 0:512], start=True, stop=True)

        # Batched loads (one DMA per tensor/chunk: per-DMA queue overhead
        # ~0.6us makes many small DMAs startup-dominant).  Critical stream
        # on sync: wq, x qs 0:512 (unblocks the qq-split prologue), wk,
        # x qs 512:1024, wv, x qs 1024:1536.
        def w_load(eng, dst, src):
            eng.dma_start(
                dst[:, :, :], src[:, :].rearrange("(k p) e -> p k e", p=128)
            )

        def x_load(eng, nb):
            eng.dma_start(
                x_sb[:, :, nb * 512:(nb + 1) * 512],
                xT[:, nb * 512:(nb + 1) * 512].rearrange(
                    "(k p) s -> p k s", p=128
                ),
            )

        def w_half(eng, dst, srcT, m):
            eng.dma_start(
                dst[:, :, m * 128:(m + 1) * 128],
                srcT[:, m * 128:(m + 1) * 128].rearrange(
                    "(k p) e -> p k e", p=128
                ),
            )

        w_half(nc.sync, wq_sb, wqT, 0)
        x_load(nc.sync, 0)
        w_half(nc.sync, wk_sb, wkT, 0)
        x_load(nc.sync, 1)
        w_load(nc.sync, wv_sb, wvT)
        w_half(nc.sync, wq_sb, wqT, 1)
        w_half(nc.sync, wk_sb, wkT, 1)
        x_load(nc.sync, 2)
        # ones column of v (softmax denominator rides the PV matmul)
        nc.gpsimd.memset(v_sb[:, :, :, HD:HD + 1], 1.0)

        # ---- emission helpers ----
        rows = [0]          # PE rows emitted so far (cost-model pacing)

        # rotating psum slots for projection/out-proj work.  pj is the
        # steady-state slot; during phase A and the tail the (then idle)
        # score tiles provide 4 more bank-aligned [128,512] slots.
        def slot_views():
            return [pj[:, 0:512], sc[0][:, 0:512], sc[0][:, 512:1024],
                    sc[1][:, 0:512], sc[1][:, 512:1024]]

        def qk_half(w_sb, dst, m, c0, slot):
            # one 256-wide column block of the q or k projection for pair m
            for k in range(KT):
                nc.tensor.matmul(
                    slot[:, 0:256],
                    w_sb[:, k, m * 128:(m + 1) * 128],
                    x_sb[:, k, c0:c0 + 256],
                    start=(k == 0),
                    stop=(k == KT - 1),
                )
            with nc.allow_low_precision("q/k rounded to bf16 for scores"):
                nc.vector.tensor_copy(dst[:, c0:c0 + 256], slot[:, 0:256])
            rows[0] += KT * 256

        def v_chunk(sbi, slot):
            for k in range(KT):
                nc.tensor.matmul(
                    slot[:, 0:256],
                    x_sb[:, k, sbi * 128:(sbi + 1) * 128],
                    wv_sb[:, k, :],
                    start=(k == 0),
                    stop=(k == KT - 1),
                )
            with nc.allow_low_precision("v rounded to bf16 for the PV matmul"):
                nc.vector.tensor_copy(
                    v_sb[:, sbi, :, 0:HD],
                    slot[:, 0:256].rearrange("p (h e) -> p h e", h=HG),
                )
            rows[0] += KT * 256

        st_cnt = [0]

        st_mid = {}

        def oproj_mid(sbi, nb):
            # one d-half of an s block of the qh0 out-projection (mid-stream,
            # single pj slot; per-nb units so the slot WAR sits between pops)
            if sbi not in st_mid:
                st_mid[sbi] = st_pool.tile([128, 1024], F16,
                                           tag=f"st{sbi % 4}", name="st")
            st = st_mid[sbi]
            for m in range(2):
                nc.tensor.matmul(
                    pj[:, 0:512],
                    ctxT[m][:, sbi * 128:(sbi + 1) * 128],
                    wo_sb[:, m, nb * 512:(nb + 1) * 512],
                    start=(m == 0),
                    stop=(m == 1),
                )
            with nc.allow_low_precision("output partial staged as f16"):
                nc.vector.tensor_copy(st[:, nb * 512:(nb + 1) * 512],
                                      pj[:, 0:512])
            if nb == 1:
                eng = nc.sync if sbi % 2 == 0 else nc.gpsimd
                eng.dma_start(out[sbi * 128:(sbi + 1) * 128, :], st[:])
            rows[0] += 2 * 512

        def oproj(sbi, slotA=None, slotB=None, tail=False):
            # both d-halves of one s block; staged f16 and stored with a
            # single DMA (per-DMA queue cost ~0.5us makes 32 stores pricey)
            i = st_cnt[0]
            st_cnt[0] += 1
            slotA = pj[:, 0:512] if slotA is None else slotA
            slotB = pj[:, 0:512] if slotB is None else slotB
            st = st_pool.tile([128, 1024], F16, tag=f"st{i % 4}", name="st")
            for nb, slot in ((0, slotA), (1, slotB)):
                for m in range(2):
                    nc.tensor.matmul(
                        slot[:, 0:512],
                        ctxT[m][:, sbi * 128:(sbi + 1) * 128],
                        wo_sb[:, m, nb * 512:(nb + 1) * 512],
                        start=(m == 0),
                        stop=(m == 1),
                    )
                with nc.allow_low_precision("output partial staged as f16"):
                    dst = st[:, nb * 512:(nb + 1) * 512]
                    if (i + nb) % 2 == 0:
                        nc.vector.tensor_copy(dst, slot[:, 0:512])
                    else:
                        nc.scalar.copy(dst, slot[:, 0:512])
            dma_eng = (nc.sync, nc.gpsimd, nc.scalar)[i % 3]
            dma_eng.dma_start(out[sbi * 128:(sbi + 1) * 128, :], st[:])
            rows[0] += 4 * 512

        def pv_step(m, qh, j, pts):
            # PSUM zeroing is lazy per 2KB zero region: start=True marks the
            # whole bank pending-zero, and each sub-tile's first touch then
            # overwrites while later touches accumulate.  So only the FIRST
            # write into each bank carries start, the LAST carries stop.
            for r in range(2):
                for qb in range(8):
                    if qb < 7:
                        dst = (pva if r == 0 else pvb)[:, qb, :]
                        first = j == 0 and qb == 0
                        stop = j == SB - 1 and qb == 6
                    else:
                        dst = pvs[:, r, :]
                        first = j == 0 and r == 0
                        stop = j == SB - 1 and r == 1
                    nc.tensor.matmul(
                        dst,
                        pts[r][:, qb * 128:(qb + 1) * 128],
                        v_sb[:, j, 2 * m + r, :],
                        start=first,
                        stop=stop,
                        skip_group_check=True,
                    )
            rows[0] += 16 * (HD + 1)

        def finalize(m, qh, last=False):
            # reciprocal of the denominator columns, then normalize-fold the
            # PSUM evacuation into ctxQ (bf16), then transpose into ctxT.
            # Streamed per q block (fold r0 on DVE, r1 on Pool, transpose
            # immediately after) so the tail drains as a pipeline.
            for r in range(2):
                pv = pva if r == 0 else pvb
                nc.vector.reciprocal(
                    rec[m][:, r, qh * 8:qh * 8 + 7],
                    pv[:, 0:7, HD:HD + 1].rearrange("p a b -> p (a b)"),
                )
                nc.vector.reciprocal(
                    rec[m][:, r, qh * 8 + 7:qh * 8 + 8],
                    pvs[:, r, HD:HD + 1],
                )
            with nc.allow_low_precision("attention context rounded to bf16"):
                for qb in range(8):
                    for r in range(2):
                        src = (pva if r == 0 else pvb)[:, qb, 0:HD] if qb < 7 \
                            else pvs[:, r, 0:HD]
                        dst = ctxQ[m][:, qh * 8 + qb, r * 64:(r + 1) * 64]
                        sca = rec[m][:, r, qh * 8 + qb:qh * 8 + qb + 1]
                        if last and r == 1:
                            # ACT is free after the last exp; its Copy-with-
                            # scale IS the normalize-fold (GPSIMD cannot
                            # touch PSUM on hardware, so only DVE/ACT can)
                            nc.scalar.activation(
                                dst, src,
                                mybir.ActivationFunctionType.Copy,
                                scale=sca,
                            )
                        else:
                            nc.vector.tensor_scalar_mul(dst, src, sca)
                    qg = qh * 8 + qb
                    if not last:
                        # transpose via the DMA XBAR (latency hides behind
                        # the still-running exp stream)
                        nc.sync.dma_start_transpose(
                            ctxT[m][:, qg * 128:(qg + 1) * 128],
                            ctxQ[m][:, qg, :],
                        )
                    else:
                        # tail: PE transpose (PE is idle; the XBAR's ~1.7us
                        # per-tile latency chain would gate the drain) and
                        # fused out-projection per s block
                        tsl = sc[1][:, qb * 64:(qb + 1) * 64].bitcast(BF16)
                        nc.tensor.matmul(
                            tsl, ctxQ[m][:, qg, :], ident_sb[:, :],
                            is_transpose=True,
                            start=(qb == 0), stop=(qb == 7),
                            skip_group_check=True,
                        )
                        rows[0] += 128
                        if qb % 2 == 0:
                            nc.scalar.copy(
                                ctxT[m][:, qg * 128:(qg + 1) * 128], tsl)
                        else:
                            nc.vector.tensor_copy(
                                ctxT[m][:, qg * 128:(qg + 1) * 128], tsl)
                        slots4 = [pj[:, 0:512], sc[0][:, 0:512],
                                  sc[0][:, 512:1024], sc[1][:, 512:1024]]
                        oproj(qg, slots4[(2 * qb) % 4],
                              slots4[(2 * qb + 1) % 4], tail=True)

        # ---- phase A: minimum needed for the first (qq-split) scores ----
        done = set()    # completed filler units, keyed for dep-forced pops
        slots = slot_views()
        phase_a = (
            [(("qT", 0, 0), qk_half, (wq_sb, qT[0], 0, 0)),
             (("qT", 0, 256), qk_half, (wq_sb, qT[0], 0, 256)),
             (("kT", 0, 0), qk_half, (wk_sb, kT[0], 0, 0))]
        )
        for i, (key, fn, args) in enumerate(phase_a):
            fn(*args, slots[i % 5])
            done.add(key)

        # late DMA stream (gpsimd): gated behind the first qT copy so it
        # doesn't dilute the critical sync-queue stream on the DMA engines.
        # DMA queue preps schedule by data deps, not engine order, so gate
        # each late DMA with a WAW dep: scribble into its destination from a
        # copy that depends on the first qT tile (ready ~9us in).
        with nc.allow_low_precision("gate scribbles, overwritten by DMAs"):
            nc.gpsimd.tensor_copy(x_sb[0:1, 0, 1536:1544], qT[0][0:1, 0:8])
            x_load(nc.gpsimd, 3)
            for m in range(2):
                nc.gpsimd.tensor_copy(wo_sb[0:1, m, 0:8], qT[0][0:1, 0:8])
                nc.gpsimd.dma_start(wo_sb[:, m, :], woT[m * 128:(m + 1) * 128, :])
            nc.gpsimd.tensor_copy(ident_sb[0:1, 0:8], qT[0][0:1, 0:8])
            nc.gpsimd.dma_start(ident_sb[:, :], identT[:, :])

        # ---- filler queue (deadline-ordered; pops may be dep-forced) ----
        fill = deque()

        def add_qk(w_sb, dst, m, c):
            t = "qT" if dst is qT[m] else "kT"
            fill.append(((t, m, c), qk_half, (w_sb, dst, m, c)))

        def add_v(j):
            fill.append((("v", j), v_chunk, (j,)))

        add_qk(wk_sb, kT[0], 0, 256)
        add_qk(wq_sb, qT[0], 0, 512)
        add_qk(wq_sb, qT[0], 0, 768)
        add_v(0)
        add_v(1)
        add_v(2)
        add_qk(wk_sb, kT[0], 0, 512)
        add_v(3)
        add_qk(wk_sb, kT[0], 0, 768)
        add_v(4)
        add_qk(wk_sb, kT[0], 0, 1024)
        add_qk(wq_sb, qT[0], 0, 1024)
        add_v(5)
        add_qk(wk_sb, kT[0], 0, 1280)
        add_qk(wq_sb, qT[0], 0, 1280)
        add_v(6)
        add_qk(wk_sb, kT[0], 0, 1536)
        add_qk(wq_sb, qT[0], 0, 1536)
        add_v(7)
        add_qk(wk_sb, kT[0], 0, 1792)
        add_qk(wq_sb, qT[0], 0, 1792)
        add_v(8)
        add_qk(wq_sb, qT[1], 1, 0)
        add_v(9)
        add_qk(wq_sb, qT[1], 1, 256)
        add_v(10)
        add_qk(wq_sb, qT[1], 1, 512)
        add_v(11)
        add_qk(wq_sb, qT[1], 1, 768)
        add_v(12)
        add_qk(wk_sb, kT[1], 1, 0)
        add_v(13)
        add_qk(wk_sb, kT[1], 1, 256)
        add_v(14)
        add_qk(wk_sb, kT[1], 1, 512)
        add_v(15)
        add_qk(wk_sb, kT[1], 1, 768)
        for c in range(1024, S, 256):
            add_qk(wq_sb, qT[1], 1, c)
            add_qk(wk_sb, kT[1], 1, c)

        # ---- main 64-step pipeline ----
        group_seq = [(0, 0), (0, 1), (1, 0), (1, 1)]
        steps = [(m, qh, j) for (m, qh) in group_seq for j in range(SB)]
        N_PRO = 4            # qq-split prologue steps (512-wide exps)

        pv_q = deque()
        pcnt = [0]
        act_ns = [0.0]       # cumulative ACT time emitted (pacing reference)
        PE_NS_PER_ROW = 1.0 / 2.4

        fill_cnt = [0]

        def pop_fill(s):
            key, fn, args = fill.popleft()
            if fn is oproj_mid:
                fn(*args)
            else:
                fn(*args, pj[:, 0:512])
            done.add(key)

        def ensure(key):
            # PE is in-order: an instruction whose operand-producing unit is
            # emitted later deadlocks the queue.  Force-pop until produced.
            while key not in done:
                assert fill, f"dependency {key} not in fill queue"
                pop_fill(0)

        def pop_pv():
            s_, m_, qh_, j_, pts = pv_q.popleft()
            ensure(("v", j_))
            pv_step(m_, qh_, j_, pts)
            if j_ == SB - 1:
                finalize(m_, qh_, last=(m_, qh_) == (1, 1))
                if (m_, qh_) == (1, 0):
                    for sbi in range(8):
                        for nb in range(2):
                            fill.append((("op", sbi, nb), oproj_mid, (sbi, nb)))


        def emit_exp(pt_ap, sc_ap, width):
            nc.scalar.activation(pt_ap, sc_ap, EXP)
            act_ns[0] += width * 0.8333 + 185.0

        # prologue: the first N_PRO steps run as two 512-wide passes — all
        # qq0 scores+exps first (they only need wq + x qs 0:512), then the
        # qq1 pass once the x qs 512:1024 DMA has landed.  This starts the
        # ACT stream ~8us earlier than a full-width first step.
        pro_pts = []
        for s in range(4):
            m, qh, j = steps[s]
            pts = []
            for r in range(2):
                pt = p_pool.tile([128, 1024], BF16,
                                 tag=f"p{pcnt[0] % NP}", name="pt")
                pcnt[0] += 1
                pts.append(pt)
            pro_pts.append(pts)
        for qq in range(2):
            for c in range(qq * 512, (qq + 1) * 512, 256):
                ensure(("qT", 0, c))
            for s in range(4):
                m, qh, j = steps[s]
                ensure(("kT", m, (j * 128 // 256) * 256))
                for r in range(2):
                    nc.tensor.matmul(
                        sc[r][:, qq * 512:(qq + 1) * 512],
                        kT[m][r * 64:r * 64 + 64, j * 128:(j + 1) * 128],
                        qT[m][r * 64:r * 64 + 64, qq * 512:(qq + 1) * 512],
                        start=True,
                        stop=True,
                    )
                    rows[0] += 512
                    emit_exp(pro_pts[s][r][:, qq * 512:(qq + 1) * 512],
                             sc[r][:, qq * 512:(qq + 1) * 512], 512)
                if qq == 1 and fill and s % 2 == 0:
                    # weave a projection pop between the second-pass scores
                    # so the step-4 scores aren't stuck behind them all
                    pop_fill(s)

        for s, (m, qh, j) in enumerate(steps):
            ensure(("kT", m, (j * 128 // 256) * 256))
            if s < N_PRO:
                pts = pro_pts[s]   # scores+exps already emitted above
            else:
                pts = []
                for c in range(qh * 1024, (qh + 1) * 1024, 256):
                    ensure(("qT", m, c))
                for r in range(2):
                    for qq in range(2):
                        nc.tensor.matmul(
                            sc[r][:, qq * 512:(qq + 1) * 512],
                            kT[m][r * 64:r * 64 + 64, j * 128:(j + 1) * 128],
                            qT[m][r * 64:r * 64 + 64,
                                  qh * 1024 + qq * 512:qh * 1024 + (qq + 1) * 512],
                            start=True,
                            stop=True,
                        )
                    rows[0] += 1024
                    pt = p_pool.tile([128, 1024], BF16,
                                     tag=f"p{pcnt[0] % NP}", name="pt")
                    pcnt[0] += 1
                    emit_exp(pt[:], sc[r][:], 1024)
                    pts.append(pt)
            pv_q.append((s, m, qh, j, pts))

            # pacing: emit PE work to track the ACT stream (rows whose PE
            # time matches cumulative ACT time), never popping a PV before
            # its exps have had a step to run (and not before wv landed).
            target = act_ns[0] * 2.4
            nfill = 0
            while pv_q or fill:
                can_pv = pv_q and pv_q[0][0] < s and s >= 6
                if can_pv and len(pv_q) > 16:
                    pop_pv()
                    continue
                if rows[0] >= target:
                    break
                if fill and nfill < 2:
                    pop_fill(s)
                    nfill += 1
                elif can_pv:
                    pop_pv()
                else:
                    break

        while pv_q:
            pop_pv()
        while fill:
            pop_fill(63)

        assert not fill
    return nc


_NC_CACHE = None


def _get_nc():
    global _NC_CACHE
    if _NC_CACHE is None:
        _NC_CACHE = build_nc()
    return _NC_CACHE


_EXEC_CACHE = None


def _get_executor():
    """Build + jit the SPMD executable once; reuse across kernel() calls.

    Mirrors concourse.bass2jax.run_bass_via_pjrt, which re-jits on every
    call (full retrace + executable reload); caching shaves seconds/call."""
    global _EXEC_CACHE
    if _EXEC_CACHE is not None:
        return _EXEC_CACHE
    import jax
    from jax.sharding import Mesh, PartitionSpec
    from jax.experimental.shard_map import shard_map
    from concourse import bass2jax as b2j

    nc = _get_nc()
    b2j.install_neuronx_cc_hook()
    assert nc.dbg_addr is None
    partition_name = (
        nc.partition_id_tensor.name if nc.partition_id_tensor is not None else None
    )

    in_names, out_names, out_avals = [], [], []
    for alloc in nc.m.functions[0].allocations:
        if not isinstance(alloc, mybir.MemoryLocationSet):
            continue
        name = alloc.memorylocations[0].name
        if alloc.kind == "ExternalInput":
            if name != partition_name:
                in_names.append(name)
        elif alloc.kind == "ExternalOutput":
            out_names.append(name)
            out_avals.append(
                jax.core.ShapedArray(
                    tuple(alloc.tensor_shape), mybir.dt.np(alloc.dtype)
                )
            )
    n_params = len(in_names)
    n_outs = len(out_avals)
    all_names = in_names + out_names
    if partition_name is not None:
        all_names = all_names + [partition_name]

    def _body(*args):
        operands = list(args)
        if partition_name is not None:
            operands.append(b2j.partition_id_tensor())
        outs = b2j._bass_exec_p.bind(
            *operands,
            out_avals=tuple(out_avals),
            in_names=tuple(all_names),
            out_names=tuple(out_names),
            lowering_input_output_aliases=(),
            sim_require_finite=True,
            sim_require_nnan=True,
            nc=nc,
        )
        return tuple(outs)

    devices = jax.devices()[:NCORES]
    mesh = Mesh(np.asarray(devices), ("core",))
    donate = tuple(range(n_params, n_params + n_outs))
    sharded = jax.jit(
        shard_map(
            _body,
            mesh=mesh,
            in_specs=(PartitionSpec("core"),) * (n_params + n_outs),
            out_specs=(PartitionSpec("core"),) * n_outs,
            check_rep=False,
        ),
        donate_argnums=donate,
        keep_unused=True,
    )
    import jax.numpy as jnp

    zero_shardings = [
        jax.sharding.NamedSharding(mesh, PartitionSpec("core"))
    ] * n_outs

    @jax.jit
    def _make_zeros():
        return tuple(
            jax.lax.with_sharding_constraint(
                jnp.zeros((NCORES * a.shape[0], *a.shape[1:]), a.dtype), sh
            )
            for a, sh in zip(out_avals, zero_shardings)
        )

    _EXEC_CACHE = {
        "sharded": sharded,
        "make_zeros": _make_zeros,
        "in_names": in_names,
        "out_names": out_names,
        "out_avals": out_avals,
    }
    return _EXEC_CACHE


def _run_spmd(in_maps):
    ex = _get_executor()
    concat_in = [
        np.concatenate([np.asarray(m[name]) for m in in_maps], axis=0)
        for name in ex["in_names"]
    ]
    concat_zeros = ex["make_zeros"]()
    out_arrs = ex["sharded"](*concat_in, *concat_zeros)
    results = []
    for c in range(NCORES):
        results.append({
            name: np.asarray(out_arrs[i]).reshape(
                NCORES, *ex["out_avals"][i].shape
            )[c]
            for i, name in enumerate(ex["out_names"])
        })
    return results


def _shard_inputs(x, Wq, Wk, Wv, Wo):
    import ml_dtypes

    scale = np.float32(1.0 / np.sqrt(HD))
    in_maps = []
    xT_b = [np.ascontiguousarray(x[b].T).astype(ml_dtypes.bfloat16) for b in range(B)]
    ident = np.eye(128, dtype=ml_dtypes.bfloat16)
    for c in range(NCORES):
        b, g = divmod(c, GROUPS)
        sl = slice(g * E, (g + 1) * E)
        in_maps.append({
            "xT": xT_b[b],
            "wqT": np.ascontiguousarray(Wq[sl, :].T * scale).astype(ml_dtypes.bfloat16),
            "wkT": np.ascontiguousarray(Wk[sl, :].T).astype(ml_dtypes.bfloat16),
            "wvT": np.ascontiguousarray(Wv[sl, :].T).astype(ml_dtypes.bfloat16),
            "woT": np.ascontiguousarray(Wo[:, sl].T).astype(ml_dtypes.bfloat16),
            "identT": ident,
        })
    return in_maps


_FAST_PATH_OK = True


def kernel(x, Wq, Wk, Wv, Wo, bo):
    global _FAST_PATH_OK
    x = np.asarray(x, dtype=np.float32)
    in_maps = _shard_inputs(
        x,
        np.asarray(Wq, dtype=np.float32),
        np.asarray(Wk, dtype=np.float32),
        np.asarray(Wv, dtype=np.float32),
        np.asarray(Wo, dtype=np.float32),
    )
    results = None
    if _FAST_PATH_OK:
        try:
            results = _run_spmd(in_maps)
        except Exception:
            _FAST_PATH_OK = False
    if results is None:
        # portable fallback: stock SPMD runner (handles native-device
        # environments and anything the cached-PJRT fast path can't)
        results = run_bass_kernel_spmd(
            _get_nc(), in_maps, list(range(NCORES))
        ).results
    bo = np.asarray(bo, dtype=np.float32)
    out = np.empty((B, S, D), dtype=np.float32)
    for b in range(B):
        acc = np.zeros((S, D), dtype=np.float64)
        for g in range(GROUPS):
            acc += results[b * GROUPS + g]["out_partial"].astype(np.float64)
        out[b] = (acc + bo.astype(np.float64)).astype(np.float32)
    return out


# revision 57
# speedup vs baseline: 1.0435x; 1.0435x over previous
"""Multi-head attention (B=2, S=2048, D=1024, H=16) on 8 Trainium2 NeuronCores.

Sharding: core c handles batch b = c//4 and head group g = c%4 (4 heads, 256
model dims).  Each core computes q/k/v projections for its heads, attention,
and a partial output projection (row-parallel over its 256 head dims); the
host sums the 4 partials per batch and adds the bias.

Layouts / engine plan (v2 — PV reoriented, softmax normalize folded):
  xT  [d, s] f32r (host pre-transposed); wq/wk/wv f32r; wo bf16.
  qT/kT [e(128 = head pair), s] f32r; scores computed transposed [ks, qs] in
  PSUM, exp on ACT -> p [ks, qs] bf16 in SBUF (ACT does nothing else, so the
  exp stream is the makespan spine at ~133us).
  PV is out[q(128), e(65)] with K=128 (lhsT = p block, rhs = v[ks, e+1] with a
  trailing ones column): the 65-col free dim runs at 1 cyc/row in bf16, and
  column 64 accumulates the softmax denominator per-q (a per-PARTITION
  scalar).  Normalization is folded into the PSUM evacuation: DVE
  reciprocal of the denominator column + tensor_scalar multiply -> ctxQ bf16.
  ctxQ [q, e-pair] -> ctxT [e-pair, q] via the XBAR dma transpose (bf16,
  SBUF->SBUF), then the output projection contracts K=128 per head pair.
  Output partials staged f16 (DVE/Pool copies) and DMA'd out.

PSUM (8 banks): sc0+sc1 [128,1024] f32 (4) + pva/pvb [128,7,65] + pvs
[128,2,65] (3, one pool each so they are bank-aligned) + pj [128,512] (1).

Emission is a 64-step pipeline (4 (m,qh) groups x 16 ks blocks): each step
emits the two score matmuls + exps, then pops PV steps and "filler" units
(qkv projection chunks, out-projection tiles) against a rows-emitted pacing
target so the PE queue tracks the ACT stream without starving it.
"""

import os
import sys

import numpy as np

for _p in ("/opt/trn_rl_repo", "/root/.axon_site/_ro/trn_rl_repo"):
    if os.path.isdir(_p) and _p not in sys.path:
        sys.path.insert(0, _p)

import bass_rust
import concourse.bass as bass
import concourse.mybir as mybir
import concourse.tile as tile
from concourse.bass_utils import run_bass_kernel_spmd
from concourse.vector_clock import ScopedClock, VectorClock
from contextlib import ExitStack
from collections import deque

F32 = mybir.dt.float32
F32R = mybir.dt.float32r
BF16 = mybir.dt.bfloat16
F16 = mybir.dt.float16
EXP = mybir.ActivationFunctionType.Exp

B = 2
S = 2048
D = 1024
H = 16
HD = 64
NCORES = 8
GROUPS = 4          # head groups (cores per batch)
HG = H // GROUPS    # heads per core = 4
E = HG * HD         # head dims per core = 256
KT = D // 128       # contraction tiles over model dim = 8
SB = S // 128       # 128-row s blocks = 16
NP = 36             # p-tile ring size (bf16 [128,1024] tiles)

_carrier_counter = [0]


def _split_multi_waits(ordered):
    """This walrus build allows one sync wait per instruction; Tile's wait
    assignment can attach several.  Hoist extras onto same-engine InstNoOp
    carriers placed immediately before the instruction."""
    for bb_name, insts in ordered.items():
        new_list = []
        for inst in insts:
            si = inst.sync_info
            waits = list(si.on_wait) if si is not None else []
            if len(waits) > 1:
                for w in waits[:-1]:
                    _carrier_counter[0] += 1
                    carrier = mybir.InstNoOp(
                        name=f"I-waitc-{_carrier_counter[0]}", ins=[], outs=[]
                    )
                    carrier.engine = inst.engine
                    carrier.sync_info = bass_rust.SyncInfo(on_wait=[w], on_update=[])
                    new_list.append(carrier)
                inst.sync_info = bass_rust.SyncInfo(
                    on_wait=[waits[-1]],
                    on_update=list(si.on_update) if si is not None else [],
                )
            new_list.append(inst)
        ordered[bb_name] = new_list


class _TileContext(tile.TileContext):
    """TileContext adapted to the one-sync-wait-per-instruction walrus."""

    def _lower_ordered_insts(self, ordered):
        _split_multi_waits(ordered)
        return super()._lower_ordered_insts(ordered)

    def _drain_and_barrier(self, tick_clock, wait_clock):
        gc = tick_clock.global_clock
        for proc in range(len(gc)):
            if gc[proc] <= 0:
                continue
            cur = VectorClock([0 if i == proc else gc[i] for i in range(len(gc))])
            nop = self.nc.sync.nop()
            wait_clock.add_sem_waits(
                nop.ins, ScopedClock({None: gc}), ScopedClock({None: cur})
            )
        drain_inst = self.nc.sync.drain()
        wait_clock.add_sem_waits(
            drain_inst.ins, ScopedClock({None: gc}), ScopedClock({None: gc.copy()})
        )
        self.nc.all_engine_barrier()
        assert self.sems is not None
        popped = self.nc._tile_sem_poison_stack.pop()
        assert popped is self._sem_poison
        self.nc.clear_and_free_semaphores(list(self.sems.allocated().values()))
        self.nc.all_engine_barrier()


def build_nc():
    nc = bass.Bass()
    xT = nc.declare_dram_parameter("xT", [D, S], BF16, isOutput=False)
    wqT = nc.declare_dram_parameter("wqT", [D, E], BF16, isOutput=False)
    wkT = nc.declare_dram_parameter("wkT", [D, E], BF16, isOutput=False)
    wvT = nc.declare_dram_parameter("wvT", [D, E], BF16, isOutput=False)
    woT = nc.declare_dram_parameter("woT", [E, D], BF16, isOutput=False)
    identT = nc.declare_dram_parameter("identT", [128, 128], BF16, isOutput=False)
    out = nc.declare_dram_parameter("out_partial", [S, D], F16, isOutput=True)

    with _TileContext(nc) as tc, ExitStack() as ctx:
        sb = ctx.enter_context(tc.tile_pool(name="sb", bufs=1))
        x_sb = sb.tile([128, KT, S], BF16, tag="x", name="x_sb")
        wq_sb = sb.tile([128, KT, E], BF16, tag="wq", name="wq_sb")
        wk_sb = sb.tile([128, KT, E], BF16, tag="wk", name="wk_sb")
        wv_sb = sb.tile([128, KT, E], BF16, tag="wv", name="wv_sb")
        wo_sb = sb.tile([128, 2, D], BF16, tag="wo", name="wo_sb")
        qT = [sb.tile([128, S], BF16, tag=f"qT{m}", name=f"qT{m}") for m in range(2)]
        kT = [sb.tile([128, S], BF16, tag=f"kT{m}", name=f"kT{m}") for m in range(2)]
        v_sb = sb.tile([128, SB, HG, HD + 1], BF16, tag="v", name="v_sb")
        ctxQ = [sb.tile([128, SB, 128], BF16, tag=f"cq{m}", name=f"cq{m}")
                for m in range(2)]
        ctxT = [sb.tile([128, S], BF16, tag=f"ct{m}", name=f"ct{m}")
                for m in range(2)]
        rec = [sb.tile([128, 2, SB], F32, tag=f"rec{m}", name=f"rec{m}")
               for m in range(2)]
        ident_sb = sb.tile([128, 128], BF16, tag="ident", name="ident_sb")

        p_pool = ctx.enter_context(tc.tile_pool(name="pp", bufs=1))
        st_pool = ctx.enter_context(tc.tile_pool(name="st", bufs=1))

        ps_sc = ctx.enter_context(tc.tile_pool(name="ps_sc", bufs=1, space="PSUM"))
        ps_pva = ctx.enter_context(tc.tile_pool(name="ps_pva", bufs=1, space="PSUM"))
        ps_pvb = ctx.enter_context(tc.tile_pool(name="ps_pvb", bufs=1, space="PSUM"))
        ps_pvs = ctx.enter_context(tc.tile_pool(name="ps_pvs", bufs=1, space="PSUM"))
        ps_pj = ctx.enter_context(tc.tile_pool(name="ps_pj", bufs=1, space="PSUM"))

        sc = [ps_sc.tile([128, 1024], F32, tag=f"sc{r}", name=f"sc{r}")
              for r in range(2)]
        pva = ps_pva.tile([128, 7, HD + 1], F32, tag="pva", name="pva")
        pvb = ps_pvb.tile([128, 7, HD + 1], F32, tag="pvb", name="pvb")
        pvs = ps_pvs.tile([128, 2, HD + 1], F32, tag="pvs", name="pvs")
        pj = ps_pj.tile([128, 512], F32, tag="pj", name="pj")

        # ---- input DMAs ----
        # Priority-interleaved so the phase-A projections start after the
        # first (wq_k, x_k) pair instead of after the whole weight load:
        # sync queue carries the critical path (wq/wk + x for qs 0:1024),
        # gpsimd the rest (wv, wo, x tail).
        # preload the Exp activation table while DMAs stream (saves the
        # 1.3us implicit table load before the first real exp)
        warm = sb.tile([1, 512], BF16, tag="warm", name="warm")
        warmf = sb.tile([1, 8], F32, tag="warmf", name="warmf")
        nc.vector.memset(warm[0:1, :], 0.0)
        nc.vector.memset(warmf[0:1, 0:8], 0.0)
        nc.scalar.activation(warmf[0:1, 0:8], warmf[0:1, 0:8], EXP)
        # PE p-state warmup: a dep-free dummy matmul train ramps the tensor
        # engine to full clock before the first projection lands (~6us in)
        for _ in range(14):
            nc.tensor.matmul(pj[0:1, 0:512], warm[0:1, 0:1],
                             warm[0:1, 0:512], start=True, stop=True)

        # Batched loads (one DMA per tensor/chunk: per-DMA queue overhead
        # ~0.6us makes many small DMAs startup-dominant).  Critical stream
        # on sync: wq, x qs 0:512 (unblocks the qq-split prologue), wk,
        # x qs 512:1024, wv, x qs 1024:1536.
        def w_load(eng, dst, src):
            eng.dma_start(
                dst[:, :, :], src[:, :].rearrange("(k p) e -> p k e", p=128)
            )

        def x_load(eng, nb):
            eng.dma_start(
                x_sb[:, :, nb * 512:(nb + 1) * 512],
                xT[:, nb * 512:(nb + 1) * 512].rearrange(
                    "(k p) s -> p k s", p=128
                ),
            )

        def w_half(eng, dst, srcT, m):
            eng.dma_start(
                dst[:, :, m * 128:(m + 1) * 128],
                srcT[:, m * 128:(m + 1) * 128].rearrange(
                    "(k p) e -> p k e", p=128
                ),
            )

        w_half(nc.sync, wq_sb, wqT, 0)
        x_load(nc.sync, 0)
        w_half(nc.sync, wk_sb, wkT, 0)
        x_load(nc.sync, 1)
        w_load(nc.sync, wv_sb, wvT)
        w_half(nc.sync, wq_sb, wqT, 1)
        w_half(nc.sync, wk_sb, wkT, 1)
        x_load(nc.sync, 2)
        # ones column of v (softmax denominator rides the PV matmul)
        nc.gpsimd.memset(v_sb[:, :, :, HD:HD + 1], 1.0)

        # ---- emission helpers ----
        rows = [0]          # PE rows emitted so far (cost-model pacing)

        # rotating psum slots for projection/out-proj work.  pj is the
        # steady-state slot; during phase A and the tail the (then idle)
        # score tiles provide 4 more bank-aligned [128,512] slots.
        def slot_views():
            return [pj[:, 0:512], sc[0][:, 0:512], sc[0][:, 512:1024],
                    sc[1][:, 0:512], sc[1][:, 512:1024]]

        def qk_half(w_sb, dst, m, c0, slot):
            # one 256-wide column block of the q or k projection for pair m
            for k in range(KT):
                nc.tensor.matmul(
                    slot[:, 0:256],
                    w_sb[:, k, m * 128:(m + 1) * 128],
                    x_sb[:, k, c0:c0 + 256],
                    start=(k == 0),
                    stop=(k == KT - 1),
                )
            with nc.allow_low_precision("q/k rounded to bf16 for scores"):
                nc.vector.tensor_copy(dst[:, c0:c0 + 256], slot[:, 0:256])
            rows[0] += KT * 256

        def v_chunk(sbi, slot):
            for k in range(KT):
                nc.tensor.matmul(
                    slot[:, 0:256],
                    x_sb[:, k, sbi * 128:(sbi + 1) * 128],
                    wv_sb[:, k, :],
                    start=(k == 0),
                    stop=(k == KT - 1),
                )
            with nc.allow_low_precision("v rounded to bf16 for the PV matmul"):
                nc.vector.tensor_copy(
                    v_sb[:, sbi, :, 0:HD],
                    slot[:, 0:256].rearrange("p (h e) -> p h e", h=HG),
                )
            rows[0] += KT * 256

        st_cnt = [0]

        st_mid = {}

        def oproj_mid(sbi, nb):
            # one d-half of an s block of the qh0 out-projection (mid-stream,
            # single pj slot; per-nb units so the slot WAR sits between pops)
            if sbi not in st_mid:
                st_mid[sbi] = st_pool.tile([128, 1024], F16,
                                           tag=f"st{sbi % 4}", name="st")
            st = st_mid[sbi]
            for m in range(2):
                nc.tensor.matmul(
                    pj[:, 0:512],
                    ctxT[m][:, sbi * 128:(sbi + 1) * 128],
                    wo_sb[:, m, nb * 512:(nb + 1) * 512],
                    start=(m == 0),
                    stop=(m == 1),
                )
            with nc.allow_low_precision("output partial staged as f16"):
                nc.vector.tensor_copy(st[:, nb * 512:(nb + 1) * 512],
                                      pj[:, 0:512])
            if nb == 1:
                eng = nc.sync if sbi % 2 == 0 else nc.gpsimd
                eng.dma_start(out[sbi * 128:(sbi + 1) * 128, :], st[:])
            rows[0] += 2 * 512

        def oproj(sbi, slotA=None, slotB=None, tail=False):
            # both d-halves of one s block; staged f16 and stored with a
            # single DMA (per-DMA queue cost ~0.5us makes 32 stores pricey)
            i = st_cnt[0]
            st_cnt[0] += 1
            slotA = pj[:, 0:512] if slotA is None else slotA
            slotB = pj[:, 0:512] if slotB is None else slotB
            st = st_pool.tile([128, 1024], F16, tag=f"st{i % 4}", name="st")
            for nb, slot in ((0, slotA), (1, slotB)):
                for m in range(2):
                    nc.tensor.matmul(
                        slot[:, 0:512],
                        ctxT[m][:, sbi * 128:(sbi + 1) * 128],
                        wo_sb[:, m, nb * 512:(nb + 1) * 512],
                        start=(m == 0),
                        stop=(m == 1),
                    )
                with nc.allow_low_precision("output partial staged as f16"):
                    dst = st[:, nb * 512:(nb + 1) * 512]
                    if (i + nb) % 2 == 0:
                        nc.vector.tensor_copy(dst, slot[:, 0:512])
                    else:
                        nc.scalar.copy(dst, slot[:, 0:512])
            dma_eng = (nc.gpsimd, nc.gpsimd, nc.sync, nc.scalar,
                       nc.sync, nc.scalar, nc.sync, nc.scalar)[i % 8]
            dma_eng.dma_start(out[sbi * 128:(sbi + 1) * 128, :], st[:])
            rows[0] += 4 * 512

        def pv_step(m, qh, j, pts):
            # PSUM zeroing is lazy per 2KB zero region: start=True marks the
            # whole bank pending-zero, and each sub-tile's first touch then
            # overwrites while later touches accumulate.  So only the FIRST
            # write into each bank carries start, the LAST carries stop.
            for r in range(2):
                for qb in range(8):
                    if qb < 7:
                        dst = (pva if r == 0 else pvb)[:, qb, :]
                        first = j == 0 and qb == 0
                        stop = j == SB - 1 and qb == 6
                    else:
                        dst = pvs[:, r, :]
                        first = j == 0 and r == 0
                        stop = j == SB - 1 and r == 1
                    nc.tensor.matmul(
                        dst,
                        pts[r][:, qb * 128:(qb + 1) * 128],
                        v_sb[:, j, 2 * m + r, :],
                        start=first,
                        stop=stop,
                        skip_group_check=True,
                    )
            rows[0] += 16 * (HD + 1)

        def finalize(m, qh, last=False):
            # reciprocal of the denominator columns, then normalize-fold the
            # PSUM evacuation into ctxQ (bf16), then transpose into ctxT.
            # Streamed per q block (fold r0 on DVE, r1 on Pool, transpose
            # immediately after) so the tail drains as a pipeline.
            for r in range(2):
                pv = pva if r == 0 else pvb
                nc.vector.reciprocal(
                    rec[m][:, r, qh * 8:qh * 8 + 7],
                    pv[:, 0:7, HD:HD + 1].rearrange("p a b -> p (a b)"),
                )
                nc.vector.reciprocal(
                    rec[m][:, r, qh * 8 + 7:qh * 8 + 8],
                    pvs[:, r, HD:HD + 1],
                )
            with nc.allow_low_precision("attention context rounded to bf16"):
                def fold(qb, r):
                    src = (pva if r == 0 else pvb)[:, qb, 0:HD] if qb < 7 \
                        else pvs[:, r, 0:HD]
                    dst = ctxQ[m][:, qh * 8 + qb, r * 64:(r + 1) * 64]
                    sca = rec[m][:, r, qh * 8 + qb:qh * 8 + qb + 1]
                    if last and r == 1:
                        # ACT is free after the last exp; its Copy-with-
                        # scale IS the normalize-fold (GPSIMD cannot touch
                        # PSUM on hardware, so only DVE/ACT can)
                        nc.scalar.activation(
                            dst, src,
                            mybir.ActivationFunctionType.Copy,
                            scale=sca,
                        )
                    else:
                        nc.vector.tensor_scalar_mul(dst, src, sca)

                if not last:
                    for qb in range(8):
                        fold(qb, 0)
                        fold(qb, 1)
                        qg = qh * 8 + qb
                        # transpose via the DMA XBAR (latency hides behind
                        # the still-running exp stream)
                        nc.sync.dma_start_transpose(
                            ctxT[m][:, qg * 128:(qg + 1) * 128],
                            ctxQ[m][:, qg, :],
                        )
                else:
                    # tail drain, stage-major: folds (DVE r0 / ACT r1),
                    # XBAR transposes (write ctxT directly; their latency
                    # pipelines under the folds), then the out-projection
                    # stream over 5 PSUM slots
                    for qb in range(8):
                        fold(qb, 0)
                        fold(qb, 1)
                        qg = qh * 8 + qb
                        nc.sync.dma_start_transpose(
                            ctxT[m][:, qg * 128:(qg + 1) * 128],
                            ctxQ[m][:, qg, :],
                        )
                    slots5 = [pj[:, 0:512], sc[0][:, 0:512],
                              sc[0][:, 512:1024], sc[1][:, 0:512],
                              sc[1][:, 512:1024]]
                    for qb in range(8):
                        oproj(qh * 8 + qb, slots5[(2 * qb) % 5],
                              slots5[(2 * qb + 1) % 5], tail=True)

        # ---- phase A: minimum needed for the first (qq-split) scores ----
        done = set()    # completed filler units, keyed for dep-forced pops
        slots = slot_views()
        phase_a = (
            [(("qT", 0, 0), qk_half, (wq_sb, qT[0], 0, 0)),
             (("qT", 0, 256), qk_half, (wq_sb, qT[0], 0, 256)),
             (("kT", 0, 0), qk_half, (wk_sb, kT[0], 0, 0))]
        )
        for i, (key, fn, args) in enumerate(phase_a):
            fn(*args, slots[i % 5])
            done.add(key)

        # late DMA stream (gpsimd): gated behind the first qT copy so it
        # doesn't dilute the critical sync-queue stream on the DMA engines.
        # DMA queue preps schedule by data deps, not engine order, so gate
        # each late DMA with a WAW dep: scribble into its destination from a
        # copy that depends on the first qT tile (ready ~9us in).
        with nc.allow_low_precision("gate scribbles, overwritten by DMAs"):
            nc.gpsimd.tensor_copy(x_sb[0:1, 0, 1536:1544], qT[0][0:1, 0:8])
            x_load(nc.gpsimd, 3)
            for m in range(2):
                nc.gpsimd.tensor_copy(wo_sb[0:1, m, 0:8], qT[0][0:1, 0:8])
                nc.gpsimd.dma_start(wo_sb[:, m, :], woT[m * 128:(m + 1) * 128, :])
            nc.gpsimd.tensor_copy(ident_sb[0:1, 0:8], qT[0][0:1, 0:8])
            nc.gpsimd.dma_start(ident_sb[:, :], identT[:, :])

        # ---- filler queue (deadline-ordered; pops may be dep-forced) ----
        fill = deque()

        def add_qk(w_sb, dst, m, c):
            t = "qT" if dst is qT[m] else "kT"
            fill.append(((t, m, c), qk_half, (w_sb, dst, m, c)))

        def add_v(j):
            fill.append((("v", j), v_chunk, (j,)))

        add_qk(wk_sb, kT[0], 0, 256)
        add_qk(wq_sb, qT[0], 0, 512)
        add_qk(wq_sb, qT[0], 0, 768)
        add_qk(wk_sb, kT[0], 0, 512)
        add_qk(wk_sb, kT[0], 0, 768)
        add_v(0)
        add_v(1)
        add_v(2)
        add_v(3)
        add_v(4)
        add_qk(wk_sb, kT[0], 0, 1024)
        add_qk(wq_sb, qT[0], 0, 1024)
        add_v(5)
        add_qk(wk_sb, kT[0], 0, 1280)
        add_qk(wq_sb, qT[0], 0, 1280)
        add_v(6)
        add_qk(wk_sb, kT[0], 0, 1536)
        add_qk(wq_sb, qT[0], 0, 1536)
        add_v(7)
        add_qk(wk_sb, kT[0], 0, 1792)
        add_qk(wq_sb, qT[0], 0, 1792)
        add_v(8)
        add_qk(wq_sb, qT[1], 1, 0)
        add_v(9)
        add_qk(wq_sb, qT[1], 1, 256)
        add_v(10)
        add_qk(wq_sb, qT[1], 1, 512)
        add_v(11)
        add_qk(wq_sb, qT[1], 1, 768)
        add_v(12)
        add_qk(wk_sb, kT[1], 1, 0)
        add_v(13)
        add_qk(wk_sb, kT[1], 1, 256)
        add_v(14)
        add_qk(wk_sb, kT[1], 1, 512)
        add_v(15)
        add_qk(wk_sb, kT[1], 1, 768)
        for c in range(1024, S, 256):
            add_qk(wq_sb, qT[1], 1, c)
            add_qk(wk_sb, kT[1], 1, c)

        # ---- main 64-step pipeline ----
        group_seq = [(0, 0), (0, 1), (1, 0), (1, 1)]
        steps = [(m, qh, j) for (m, qh) in group_seq for j in range(SB)]
        N_PRO = 4            # qq-split prologue steps (512-wide exps)

        pv_q = deque()
        pcnt = [0]
        act_ns = [0.0]       # cumulative ACT time emitted (pacing reference)
        PE_NS_PER_ROW = 1.0 / 2.4

        fill_cnt = [0]

        def pop_fill(s):
            key, fn, args = fill.popleft()
            if fn is oproj_mid:
                fn(*args)
            else:
                fn(*args, pj[:, 0:512])
            done.add(key)

        def ensure(key):
            # PE is in-order: an instruction whose operand-producing unit is
            # emitted later deadlocks the queue.  Force-pop until produced.
            while key not in done:
                assert fill, f"dependency {key} not in fill queue"
                pop_fill(0)

        def pop_pv():
            s_, m_, qh_, j_, pts = pv_q.popleft()
            ensure(("v", j_))
            pv_step(m_, qh_, j_, pts)
            if j_ == SB - 1:
                finalize(m_, qh_, last=(m_, qh_) == (1, 1))
                if (m_, qh_) == (1, 0):
                    for sbi in range(8):
                        for nb in range(2):
                            fill.append((("op", sbi, nb), oproj_mid, (sbi, nb)))


        def emit_exp(pt_ap, sc_ap, width):
            nc.scalar.activation(pt_ap, sc_ap, EXP)
            act_ns[0] += width * 0.8333 + 185.0

        # prologue: the first N_PRO steps run as two 512-wide passes — all
        # qq0 scores+exps first (they only need wq + x qs 0:512), then the
        # qq1 pass once the x qs 512:1024 DMA has landed.  This starts the
        # ACT stream ~8us earlier than a full-width first step.
        pro_pts = []
        for s in range(N_PRO):
            m, qh, j = steps[s]
            pts = []
            for r in range(2):
                pt = p_pool.tile([128, 1024], BF16,
                                 tag=f"p{pcnt[0] % NP}", name="pt")
                pcnt[0] += 1
                pts.append(pt)
            pro_pts.append(pts)
        for qq in range(2):
            for c in range(qq * 512, (qq + 1) * 512, 256):
                ensure(("qT", 0, c))
            for s in range(N_PRO):
                m, qh, j = steps[s]
                ensure(("kT", m, (j * 128 // 256) * 256))
                for r in range(2):
                    nc.tensor.matmul(
                        sc[r][:, qq * 512:(qq + 1) * 512],
                        kT[m][r * 64:r * 64 + 64, j * 128:(j + 1) * 128],
                        qT[m][r * 64:r * 64 + 64, qq * 512:(qq + 1) * 512],
                        start=True,
                        stop=True,
                    )
                    rows[0] += 512
                    emit_exp(pro_pts[s][r][:, qq * 512:(qq + 1) * 512],
                             sc[r][:, qq * 512:(qq + 1) * 512], 512)
                if qq == 1 and fill:
                    # weave projection pops between the second-pass scores
                    # so the step-4 scores aren't stuck behind them all
                    pop_fill(s)
                    if fill:
                        pop_fill(s)

        for s, (m, qh, j) in enumerate(steps):
            ensure(("kT", m, (j * 128 // 256) * 256))
            if s < N_PRO:
                pts = pro_pts[s]   # scores+exps already emitted above
            else:
                pts = []
                for c in range(qh * 1024, (qh + 1) * 1024, 256):
                    ensure(("qT", m, c))
                for r in range(2):
                    for qq in range(2):
                        nc.tensor.matmul(
                            sc[r][:, qq * 512:(qq + 1) * 512],
                            kT[m][r * 64:r * 64 + 64, j * 128:(j + 1) * 128],
                            qT[m][r * 64:r * 64 + 64,
                                  qh * 1024 + qq * 512:qh * 1024 + (qq + 1) * 512],
                            start=True,
                            stop=True,
                        )
                    rows[0] += 1024
                    pt = p_pool.tile([128, 1024], BF16,
                                     tag=f"p{pcnt[0] % NP}", name="pt")
                    pcnt[0] += 1
                    emit_exp(pt[:], sc[r][:], 1024)
                    pts.append(pt)
            pv_q.append((s, m, qh, j, pts))

            # pacing: emit PE work to track the ACT stream (rows whose PE
            # time matches cumulative ACT time), never popping a PV before
            # its exps have had a step to run (and not before wv landed).
            target = act_ns[0] * 2.25
            nfill = 0
            while pv_q or fill:
                can_pv = pv_q and pv_q[0][0] < s and s >= 6
                if can_pv and len(pv_q) > 16:
                    pop_pv()
                    continue
                if rows[0] >= target:
                    break
                if fill and nfill < 2:
                    pop_fill(s)
                    nfill += 1
                elif can_pv:
                    pop_pv()
                else:
                    break

        while pv_q:
            pop_pv()
        while fill:
            pop_fill(63)

        assert not fill
    return nc


_NC_CACHE = None


def _get_nc():
    global _NC_CACHE
    if _NC_CACHE is None:
        _NC_CACHE = build_nc()
    return _NC_CACHE


_EXEC_CACHE = None


def _get_executor():
    """Build + jit the SPMD executable once; reuse across kernel() calls.

    Mirrors concourse.bass2jax.run_bass_via_pjrt, which re-jits on every
    call (full retrace + executable reload); caching shaves seconds/call."""
    global _EXEC_CACHE
    if _EXEC_CACHE is not None:
        return _EXEC_CACHE
    import jax
    from jax.sharding import Mesh, PartitionSpec
    from jax.experimental.shard_map import shard_map
    from concourse import bass2jax as b2j

    nc = _get_nc()
    b2j.install_neuronx_cc_hook()
    assert nc.dbg_addr is None
    partition_name = (
        nc.partition_id_tensor.name if nc.partition_id_tensor is not None else None
    )

    in_names, out_names, out_avals = [], [], []
    for alloc in nc.m.functions[0].allocations:
        if not isinstance(alloc, mybir.MemoryLocationSet):
            continue
        name = alloc.memorylocations[0].name
        if alloc.kind == "ExternalInput":
            if name != partition_name:
                in_names.append(name)
        elif alloc.kind == "ExternalOutput":
            out_names.append(name)
            out_avals.append(
                jax.core.ShapedArray(
                    tuple(alloc.tensor_shape), mybir.dt.np(alloc.dtype)
                )
            )
    n_params = len(in_names)
    n_outs = len(out_avals)
    all_names = in_names + out_names
    if partition_name is not None:
        all_names = all_names + [partition_name]

    def _body(*args):
        operands = list(args)
        if partition_name is not None:
            operands.append(b2j.partition_id_tensor())
        outs = b2j._bass_exec_p.bind(
            *operands,
            out_avals=tuple(out_avals),
            in_names=tuple(all_names),
            out_names=tuple(out_names),
            lowering_input_output_aliases=(),
            sim_require_finite=True,
            sim_require_nnan=True,
            nc=nc,
        )
        return tuple(outs)

    devices = jax.devices()[:NCORES]
    mesh = Mesh(np.asarray(devices), ("core",))
    donate = tuple(range(n_params, n_params + n_outs))
    sharded = jax.jit(
        shard_map(
            _body,
            mesh=mesh,
            in_specs=(PartitionSpec("core"),) * (n_params + n_outs),
            out_specs=(PartitionSpec("core"),) * n_outs,
            check_rep=False,
        ),
        donate_argnums=donate,
        keep_unused=True,
    )
    import jax.numpy as jnp

    zero_shardings = [
        jax.sharding.NamedSharding(mesh, PartitionSpec("core"))
    ] * n_outs

    @jax.jit
    def _make_zeros():
        return tuple(
            jax.lax.with_sharding_constraint(
                jnp.zeros((NCORES * a.shape[0], *a.shape[1:]), a.dtype), sh
            )
            for a, sh in zip(out_avals, zero_shardings)
        )

    _EXEC_CACHE = {
        "sharded": sharded,
        "make_zeros": _make_zeros,
        "in_names": in_names,
        "out_names": out_names,
        "out_avals": out_avals,
    }
    return _EXEC_CACHE


def _run_spmd(in_maps):
    ex = _get_executor()
    concat_in = [
        np.concatenate([np.asarray(m[name]) for m in in_maps], axis=0)
        for name in ex["in_names"]
    ]
    concat_zeros = ex["make_zeros"]()
    out_arrs = ex["sharded"](*concat_in, *concat_zeros)
    results = []
    for c in range(NCORES):
        results.append({
            name: np.asarray(out_arrs[i]).reshape(
                NCORES, *ex["out_avals"][i].shape
            )[c]
            for i, name in enumerate(ex["out_names"])
        })
    return results


def _shard_inputs(x, Wq, Wk, Wv, Wo):
    import ml_dtypes

    scale = np.float32(1.0 / np.sqrt(HD))
    in_maps = []
    xT_b = [np.ascontiguousarray(x[b].T).astype(ml_dtypes.bfloat16) for b in range(B)]
    ident = np.eye(128, dtype=ml_dtypes.bfloat16)
    for c in range(NCORES):
        b, g = divmod(c, GROUPS)
        sl = slice(g * E, (g + 1) * E)
        in_maps.append({
            "xT": xT_b[b],
            "wqT": np.ascontiguousarray(Wq[sl, :].T * scale).astype(ml_dtypes.bfloat16),
            "wkT": np.ascontiguousarray(Wk[sl, :].T).astype(ml_dtypes.bfloat16),
            "wvT": np.ascontiguousarray(Wv[sl, :].T).astype(ml_dtypes.bfloat16),
            "woT": np.ascontiguousarray(Wo[:, sl].T).astype(ml_dtypes.bfloat16),
            "identT": ident,
        })
    return in_maps


_FAST_PATH_OK = True


def kernel(x, Wq, Wk, Wv, Wo, bo):
    global _FAST_PATH_OK
    x = np.asarray(x, dtype=np.float32)
    in_maps = _shard_inputs(
        x,
        np.asarray(Wq, dtype=np.float32),
        np.asarray(Wk, dtype=np.float32),
        np.asarray(Wv, dtype=np.float32),
        np.asarray(Wo, dtype=np.float32),
    )
    results = None
    if _FAST_PATH_OK:
        try:
            results = _run_spmd(in_maps)
        except Exception:
            _FAST_PATH_OK = False
    if results is None:
        # portable fallback: stock SPMD runner (handles native-device
        # environments and anything the cached-PJRT fast path can't)
        results = run_bass_kernel_spmd(
            _get_nc(), in_maps, list(range(NCORES))
        ).results
    bo = np.asarray(bo, dtype=np.float32)
    out = np.empty((B, S, D), dtype=np.float32)
    for b in range(B):
        acc = np.zeros((S, D), dtype=np.float64)
        for g in range(GROUPS):
            acc += results[b * GROUPS + g]["out_partial"].astype(np.float64)
        out[b] = (acc + bo.astype(np.float64)).astype(np.float32)
    return out


# revision 64
# speedup vs baseline: 1.0557x; 1.0117x over previous
"""Multi-head attention (B=2, S=2048, D=1024, H=16) on 8 Trainium2 NeuronCores.

Sharding: core c handles batch b = c//4 and head group g = c%4 (4 heads, 256
model dims).  Each core computes q/k/v projections for its heads, attention,
and a partial output projection (row-parallel over its 256 head dims); the
host sums the 4 partials per batch and adds the bias.

Layouts / engine plan (v2 — PV reoriented, softmax normalize folded):
  xT  [d, s] f32r (host pre-transposed); wq/wk/wv f32r; wo bf16.
  qT/kT [e(128 = head pair), s] f32r; scores computed transposed [ks, qs] in
  PSUM, exp on ACT -> p [ks, qs] bf16 in SBUF (ACT does nothing else, so the
  exp stream is the makespan spine at ~133us).
  PV is out[q(128), e(65)] with K=128 (lhsT = p block, rhs = v[ks, e+1] with a
  trailing ones column): the 65-col free dim runs at 1 cyc/row in bf16, and
  column 64 accumulates the softmax denominator per-q (a per-PARTITION
  scalar).  Normalization is folded into the PSUM evacuation: DVE
  reciprocal of the denominator column + tensor_scalar multiply -> ctxQ bf16.
  ctxQ [q, e-pair] -> ctxT [e-pair, q] via the XBAR dma transpose (bf16,
  SBUF->SBUF), then the output projection contracts K=128 per head pair.
  Output partials staged f16 (DVE/Pool copies) and DMA'd out.

PSUM (8 banks): sc0+sc1 [128,1024] f32 (4) + pva/pvb [128,7,65] + pvs
[128,2,65] (3, one pool each so they are bank-aligned) + pj [128,512] (1).

Emission is a 64-step pipeline (4 (m,qh) groups x 16 ks blocks): each step
emits the two score matmuls + exps, then pops PV steps and "filler" units
(qkv projection chunks, out-projection tiles) against a rows-emitted pacing
target so the PE queue tracks the ACT stream without starving it.
"""

import os
import sys

import numpy as np

for _p in ("/opt/trn_rl_repo", "/root/.axon_site/_ro/trn_rl_repo"):
    if os.path.isdir(_p) and _p not in sys.path:
        sys.path.insert(0, _p)

import bass_rust
import concourse.bass as bass
import concourse.mybir as mybir
import concourse.tile as tile
from concourse.bass_utils import run_bass_kernel_spmd
from concourse.vector_clock import ScopedClock, VectorClock
from contextlib import ExitStack
from collections import deque

F32 = mybir.dt.float32
F32R = mybir.dt.float32r
BF16 = mybir.dt.bfloat16
F16 = mybir.dt.float16
EXP = mybir.ActivationFunctionType.Exp

B = 2
S = 2048
D = 1024
H = 16
HD = 64
NCORES = 8
GROUPS = 4          # head groups (cores per batch)
HG = H // GROUPS    # heads per core = 4
E = HG * HD         # head dims per core = 256
KT = D // 128       # contraction tiles over model dim = 8
SB = S // 128       # 128-row s blocks = 16
NP = 36             # p-tile ring size (bf16 [128,1024] tiles)

_carrier_counter = [0]


def _split_multi_waits(ordered):
    """This walrus build allows one sync wait per instruction; Tile's wait
    assignment can attach several.  Hoist extras onto same-engine InstNoOp
    carriers placed immediately before the instruction."""
    for bb_name, insts in ordered.items():
        new_list = []
        for inst in insts:
            si = inst.sync_info
            waits = list(si.on_wait) if si is not None else []
            if len(waits) > 1:
                for w in waits[:-1]:
                    _carrier_counter[0] += 1
                    carrier = mybir.InstNoOp(
                        name=f"I-waitc-{_carrier_counter[0]}", ins=[], outs=[]
                    )
                    carrier.engine = inst.engine
                    carrier.sync_info = bass_rust.SyncInfo(on_wait=[w], on_update=[])
                    new_list.append(carrier)
                inst.sync_info = bass_rust.SyncInfo(
                    on_wait=[waits[-1]],
                    on_update=list(si.on_update) if si is not None else [],
                )
            new_list.append(inst)
        ordered[bb_name] = new_list


class _TileContext(tile.TileContext):
    """TileContext adapted to the one-sync-wait-per-instruction walrus."""

    def _lower_ordered_insts(self, ordered):
        _split_multi_waits(ordered)
        return super()._lower_ordered_insts(ordered)

    def _drain_and_barrier(self, tick_clock, wait_clock):
        gc = tick_clock.global_clock
        for proc in range(len(gc)):
            if gc[proc] <= 0:
                continue
            cur = VectorClock([0 if i == proc else gc[i] for i in range(len(gc))])
            nop = self.nc.sync.nop()
            wait_clock.add_sem_waits(
                nop.ins, ScopedClock({None: gc}), ScopedClock({None: cur})
            )
        drain_inst = self.nc.sync.drain()
        wait_clock.add_sem_waits(
            drain_inst.ins, ScopedClock({None: gc}), ScopedClock({None: gc.copy()})
        )
        self.nc.all_engine_barrier()
        assert self.sems is not None
        popped = self.nc._tile_sem_poison_stack.pop()
        assert popped is self._sem_poison
        self.nc.clear_and_free_semaphores(list(self.sems.allocated().values()))
        self.nc.all_engine_barrier()


def build_nc():
    nc = bass.Bass()
    xT = nc.declare_dram_parameter("xT", [D, S], BF16, isOutput=False)
    wqT = nc.declare_dram_parameter("wqT", [D, E], BF16, isOutput=False)
    wkT = nc.declare_dram_parameter("wkT", [D, E], BF16, isOutput=False)
    wvT = nc.declare_dram_parameter("wvT", [D, E], BF16, isOutput=False)
    woT = nc.declare_dram_parameter("woT", [E, D], BF16, isOutput=False)
    out = nc.declare_dram_parameter("out_partial", [S, D], F16, isOutput=True)

    with _TileContext(nc) as tc, ExitStack() as ctx:
        sb = ctx.enter_context(tc.tile_pool(name="sb", bufs=1))
        x_sb = sb.tile([128, KT, S], BF16, tag="x", name="x_sb")
        wq_sb = sb.tile([128, KT, E], BF16, tag="wq", name="wq_sb")
        wk_sb = sb.tile([128, KT, E], BF16, tag="wk", name="wk_sb")
        wv_sb = sb.tile([128, KT, E], BF16, tag="wv", name="wv_sb")
        wo_sb = sb.tile([128, 2, D], BF16, tag="wo", name="wo_sb")
        qT = [sb.tile([128, S], BF16, tag=f"qT{m}", name=f"qT{m}") for m in range(2)]
        kT = [sb.tile([128, S], BF16, tag=f"kT{m}", name=f"kT{m}") for m in range(2)]
        v_sb = sb.tile([128, SB, HG, HD + 1], BF16, tag="v", name="v_sb")
        ctxQ = [sb.tile([128, SB, 128], BF16, tag=f"cq{m}", name=f"cq{m}")
                for m in range(2)]
        ctxT = [sb.tile([128, S], BF16, tag=f"ct{m}", name=f"ct{m}")
                for m in range(2)]
        rec = [sb.tile([128, 2, SB], F32, tag=f"rec{m}", name=f"rec{m}")
               for m in range(2)]

        p_pool = ctx.enter_context(tc.tile_pool(name="pp", bufs=1))
        st_pool = ctx.enter_context(tc.tile_pool(name="st", bufs=1))

        ps_sc = ctx.enter_context(tc.tile_pool(name="ps_sc", bufs=1, space="PSUM"))
        ps_pva = ctx.enter_context(tc.tile_pool(name="ps_pva", bufs=1, space="PSUM"))
        ps_pvb = ctx.enter_context(tc.tile_pool(name="ps_pvb", bufs=1, space="PSUM"))
        ps_pvs = ctx.enter_context(tc.tile_pool(name="ps_pvs", bufs=1, space="PSUM"))
        ps_pj = ctx.enter_context(tc.tile_pool(name="ps_pj", bufs=1, space="PSUM"))

        sc = [ps_sc.tile([128, 1024], F32, tag=f"sc{r}", name=f"sc{r}")
              for r in range(2)]
        pva = ps_pva.tile([128, 7, HD + 1], F32, tag="pva", name="pva")
        pvb = ps_pvb.tile([128, 7, HD + 1], F32, tag="pvb", name="pvb")
        pvs = ps_pvs.tile([128, 2, HD + 1], F32, tag="pvs", name="pvs")
        pj = ps_pj.tile([128, 512], F32, tag="pj", name="pj")

        # ---- input DMAs ----
        # Priority-interleaved so the phase-A projections start after the
        # first (wq_k, x_k) pair instead of after the whole weight load:
        # sync queue carries the critical path (wq/wk + x for qs 0:1024),
        # gpsimd the rest (wv, wo, x tail).
        # preload the Exp activation table while DMAs stream (saves the
        # 1.3us implicit table load before the first real exp)
        warm = sb.tile([1, 512], BF16, tag="warm", name="warm")
        warmf = sb.tile([1, 8], F32, tag="warmf", name="warmf")
        nc.vector.memset(warm[0:1, :], 0.0)
        nc.vector.memset(warmf[0:1, 0:8], 0.0)
        nc.scalar.activation(warmf[0:1, 0:8], warmf[0:1, 0:8], EXP)
        # PE p-state warmup: a dep-free dummy matmul train ramps the tensor
        # engine to full clock before the first projection lands (~6us in)
        for _ in range(14):
            nc.tensor.matmul(pj[0:1, 0:512], warm[0:1, 0:1],
                             warm[0:1, 0:512], start=True, stop=True)

        # Batched loads (one DMA per tensor/chunk: per-DMA queue overhead
        # ~0.6us makes many small DMAs startup-dominant).  Critical stream
        # on sync: wq, x qs 0:512 (unblocks the qq-split prologue), wk,
        # x qs 512:1024, wv, x qs 1024:1536.
        def w_load(eng, dst, src):
            eng.dma_start(
                dst[:, :, :], src[:, :].rearrange("(k p) e -> p k e", p=128)
            )

        def x_load(eng, nb):
            eng.dma_start(
                x_sb[:, :, nb * 512:(nb + 1) * 512],
                xT[:, nb * 512:(nb + 1) * 512].rearrange(
                    "(k p) s -> p k s", p=128
                ),
            )

        def w_half(eng, dst, srcT, m):
            eng.dma_start(
                dst[:, :, m * 128:(m + 1) * 128],
                srcT[:, m * 128:(m + 1) * 128].rearrange(
                    "(k p) e -> p k e", p=128
                ),
            )

        w_half(nc.sync, wq_sb, wqT, 0)
        x_load(nc.sync, 0)
        w_half(nc.sync, wk_sb, wkT, 0)
        x_load(nc.sync, 1)
        w_load(nc.sync, wv_sb, wvT)
        w_half(nc.sync, wq_sb, wqT, 1)
        w_half(nc.sync, wk_sb, wkT, 1)
        x_load(nc.sync, 2)
        # ones column of v (softmax denominator rides the PV matmul)
        nc.gpsimd.memset(v_sb[:, :, :, HD:HD + 1], 1.0)

        # ---- emission helpers ----
        rows = [0]          # PE rows emitted so far (cost-model pacing)

        # rotating psum slots for projection/out-proj work.  pj is the
        # steady-state slot; during phase A and the tail the (then idle)
        # score tiles provide 4 more bank-aligned [128,512] slots.
        def slot_views():
            return [pj[:, 0:512], sc[0][:, 0:512], sc[0][:, 512:1024],
                    sc[1][:, 0:512], sc[1][:, 512:1024]]

        def qk_half(w_sb, dst, m, c0, slot):
            # one 256-wide column block of the q or k projection for pair m
            for k in range(KT):
                nc.tensor.matmul(
                    slot[:, 0:256],
                    w_sb[:, k, m * 128:(m + 1) * 128],
                    x_sb[:, k, c0:c0 + 256],
                    start=(k == 0),
                    stop=(k == KT - 1),
                )
            with nc.allow_low_precision("q/k rounded to bf16 for scores"):
                nc.vector.tensor_copy(dst[:, c0:c0 + 256], slot[:, 0:256])
            rows[0] += KT * 256

        def v_chunk(sbi, slot):
            for k in range(KT):
                nc.tensor.matmul(
                    slot[:, 0:256],
                    x_sb[:, k, sbi * 128:(sbi + 1) * 128],
                    wv_sb[:, k, :],
                    start=(k == 0),
                    stop=(k == KT - 1),
                )
            with nc.allow_low_precision("v rounded to bf16 for the PV matmul"):
                nc.vector.tensor_copy(
                    v_sb[:, sbi, :, 0:HD],
                    slot[:, 0:256].rearrange("p (h e) -> p h e", h=HG),
                )
            rows[0] += KT * 256

        st_cnt = [0]

        st_mid = {}

        def oproj_mid(sbi, nb):
            # one d-half of an s block of the qh0 out-projection (mid-stream,
            # single pj slot; per-nb units so the slot WAR sits between pops)
            if sbi not in st_mid:
                st_mid[sbi] = st_pool.tile([128, 1024], F16,
                                           tag=f"st{sbi % 4}", name="st")
            st = st_mid[sbi]
            for m in range(2):
                nc.tensor.matmul(
                    pj[:, 0:512],
                    ctxT[m][:, sbi * 128:(sbi + 1) * 128],
                    wo_sb[:, m, nb * 512:(nb + 1) * 512],
                    start=(m == 0),
                    stop=(m == 1),
                )
            with nc.allow_low_precision("output partial staged as f16"):
                nc.vector.tensor_copy(st[:, nb * 512:(nb + 1) * 512],
                                      pj[:, 0:512])
            if nb == 1:
                eng = nc.sync if sbi % 2 == 0 else nc.gpsimd
                eng.dma_start(out[sbi * 128:(sbi + 1) * 128, :], st[:])
            rows[0] += 2 * 512

        def oproj(sbi, slotA=None, slotB=None, tail=False):
            # both d-halves of one s block; staged f16 and stored with a
            # single DMA (per-DMA queue cost ~0.5us makes 32 stores pricey)
            i = st_cnt[0]
            st_cnt[0] += 1
            slotA = pj[:, 0:512] if slotA is None else slotA
            slotB = pj[:, 0:512] if slotB is None else slotB
            st = st_pool.tile([128, 1024], F16, tag=f"st{i % 4}", name="st")
            for nb, slot in ((0, slotA), (1, slotB)):
                for m in range(2):
                    nc.tensor.matmul(
                        slot[:, 0:512],
                        ctxT[m][:, sbi * 128:(sbi + 1) * 128],
                        wo_sb[:, m, nb * 512:(nb + 1) * 512],
                        start=(m == 0),
                        stop=(m == 1),
                    )
                with nc.allow_low_precision("output partial staged as f16"):
                    dst = st[:, nb * 512:(nb + 1) * 512]
                    if (i + nb) % 2 == 0:
                        nc.vector.tensor_copy(dst, slot[:, 0:512])
                    else:
                        nc.scalar.copy(dst, slot[:, 0:512])
            dma_eng = (nc.sync, nc.gpsimd, nc.scalar)[i % 3]
            dma_eng.dma_start(out[sbi * 128:(sbi + 1) * 128, :], st[:])
            rows[0] += 4 * 512

        def pv_step(m, qh, j, pts):
            # PSUM zeroing is lazy per 2KB zero region: start=True marks the
            # whole bank pending-zero, and each sub-tile's first touch then
            # overwrites while later touches accumulate.  So only the FIRST
            # write into each bank carries start, the LAST carries stop.
            for r in range(2):
                for qb in range(8):
                    if qb < 7:
                        dst = (pva if r == 0 else pvb)[:, qb, :]
                        first = j == 0 and qb == 0
                        stop = j == SB - 1 and qb == 6
                    else:
                        dst = pvs[:, r, :]
                        first = j == 0 and r == 0
                        stop = j == SB - 1 and r == 1
                    nc.tensor.matmul(
                        dst,
                        pts[r][:, qb * 128:(qb + 1) * 128],
                        v_sb[:, j, 2 * m + r, :],
                        start=first,
                        stop=stop,
                        skip_group_check=True,
                    )
            rows[0] += 16 * (HD + 1)

        def finalize(m, qh, last=False):
            # reciprocal of the denominator columns, then normalize-fold the
            # PSUM evacuation into ctxQ (bf16), then transpose into ctxT.
            # Streamed per q block (fold r0 on DVE, r1 on Pool, transpose
            # immediately after) so the tail drains as a pipeline.
            for r in range(2):
                pv = pva if r == 0 else pvb
                nc.vector.reciprocal(
                    rec[m][:, r, qh * 8:qh * 8 + 7],
                    pv[:, 0:7, HD:HD + 1].rearrange("p a b -> p (a b)"),
                )
                nc.vector.reciprocal(
                    rec[m][:, r, qh * 8 + 7:qh * 8 + 8],
                    pvs[:, r, HD:HD + 1],
                )
            with nc.allow_low_precision("attention context rounded to bf16"):
                def fold(qb, r):
                    src = (pva if r == 0 else pvb)[:, qb, 0:HD] if qb < 7 \
                        else pvs[:, r, 0:HD]
                    dst = ctxQ[m][:, qh * 8 + qb, r * 64:(r + 1) * 64]
                    sca = rec[m][:, r, qh * 8 + qb:qh * 8 + qb + 1]
                    if last and r == 1:
                        # ACT is free after the last exp; its Copy-with-
                        # scale IS the normalize-fold (GPSIMD cannot touch
                        # PSUM on hardware, so only DVE/ACT can)
                        nc.scalar.activation(
                            dst, src,
                            mybir.ActivationFunctionType.Copy,
                            scale=sca,
                        )
                    else:
                        nc.vector.tensor_scalar_mul(dst, src, sca)

                if not last:
                    for qb in range(8):
                        fold(qb, 0)
                        fold(qb, 1)
                        qg = qh * 8 + qb
                        # transpose via the DMA XBAR (latency hides behind
                        # the still-running exp stream)
                        nc.sync.dma_start_transpose(
                            ctxT[m][:, qg * 128:(qg + 1) * 128],
                            ctxQ[m][:, qg, :],
                        )
                else:
                    # tail drain, stage-major: folds (DVE r0 / ACT r1),
                    # XBAR transposes (write ctxT directly; their latency
                    # pipelines under the folds), then the out-projection
                    # stream over 5 PSUM slots
                    for qb in range(8):
                        fold(qb, 0)
                        fold(qb, 1)
                        qg = qh * 8 + qb
                        nc.sync.dma_start_transpose(
                            ctxT[m][:, qg * 128:(qg + 1) * 128],
                            ctxQ[m][:, qg, :],
                        )
                    slots5 = [pj[:, 0:512], sc[0][:, 0:512],
                              sc[0][:, 512:1024], sc[1][:, 0:512],
                              sc[1][:, 512:1024]]
                    for qb in range(8):
                        oproj(qh * 8 + qb, slots5[(2 * qb) % 5],
                              slots5[(2 * qb + 1) % 5], tail=True)

        # ---- phase A: minimum needed for the first (qq-split) scores ----
        done = set()    # completed filler units, keyed for dep-forced pops
        slots = slot_views()
        phase_a = (
            [(("qT", 0, 0), qk_half, (wq_sb, qT[0], 0, 0)),
             (("qT", 0, 256), qk_half, (wq_sb, qT[0], 0, 256)),
             (("kT", 0, 0), qk_half, (wk_sb, kT[0], 0, 0))]
        )
        for i, (key, fn, args) in enumerate(phase_a):
            fn(*args, slots[i % 5])
            done.add(key)

        # late DMA stream (gpsimd): gated behind the first qT copy so it
        # doesn't dilute the critical sync-queue stream on the DMA engines.
        # DMA queue preps schedule by data deps, not engine order, so gate
        # each late DMA with a WAW dep: scribble into its destination from a
        # copy that depends on the first qT tile (ready ~9us in).
        with nc.allow_low_precision("gate scribbles, overwritten by DMAs"):
            nc.gpsimd.tensor_copy(x_sb[0:1, 0, 1536:1544], qT[0][0:1, 0:8])
            x_load(nc.gpsimd, 3)
            for m in range(2):
                nc.gpsimd.tensor_copy(wo_sb[0:1, m, 0:8], qT[0][0:1, 0:8])
                nc.gpsimd.dma_start(wo_sb[:, m, :], woT[m * 128:(m + 1) * 128, :])

        # ---- filler queue (deadline-ordered; pops may be dep-forced) ----
        fill = deque()

        def add_qk(w_sb, dst, m, c):
            t = "qT" if dst is qT[m] else "kT"
            fill.append(((t, m, c), qk_half, (w_sb, dst, m, c)))

        def add_v(j):
            fill.append((("v", j), v_chunk, (j,)))

        add_qk(wk_sb, kT[0], 0, 256)
        add_qk(wq_sb, qT[0], 0, 512)
        add_qk(wq_sb, qT[0], 0, 768)
        add_qk(wk_sb, kT[0], 0, 512)
        add_qk(wk_sb, kT[0], 0, 768)
        add_v(0)
        add_v(1)
        add_qk(wk_sb, kT[0], 0, 1024)
        add_qk(wq_sb, qT[0], 0, 1024)
        add_v(2)
        add_v(3)
        add_qk(wk_sb, kT[0], 0, 1280)
        add_qk(wq_sb, qT[0], 0, 1280)
        add_v(4)
        add_qk(wk_sb, kT[0], 0, 1536)
        add_qk(wq_sb, qT[0], 0, 1536)
        add_v(5)
        add_qk(wk_sb, kT[0], 0, 1792)
        add_qk(wq_sb, qT[0], 0, 1792)
        add_v(6)
        add_v(7)
        add_v(8)
        add_qk(wq_sb, qT[1], 1, 0)
        add_v(9)
        add_qk(wq_sb, qT[1], 1, 256)
        add_v(10)
        add_qk(wq_sb, qT[1], 1, 512)
        add_v(11)
        add_qk(wq_sb, qT[1], 1, 768)
        add_v(12)
        add_qk(wk_sb, kT[1], 1, 0)
        add_v(13)
        add_qk(wk_sb, kT[1], 1, 256)
        add_v(14)
        add_qk(wk_sb, kT[1], 1, 512)
        add_v(15)
        add_qk(wk_sb, kT[1], 1, 768)
        for c in range(1024, S, 256):
            add_qk(wq_sb, qT[1], 1, c)
            add_qk(wk_sb, kT[1], 1, c)

        # ---- main 64-step pipeline ----
        group_seq = [(0, 0), (0, 1), (1, 0), (1, 1)]
        steps = [(m, qh, j) for (m, qh) in group_seq for j in range(SB)]
        N_PRO = 4            # qq-split prologue steps (512-wide exps)

        pv_q = deque()
        pcnt = [0]
        act_ns = [0.0]       # cumulative ACT time emitted (pacing reference)
        PE_NS_PER_ROW = 1.0 / 2.4

        fill_cnt = [0]

        def pop_fill(s):
            key, fn, args = fill.popleft()
            if fn is oproj_mid:
                fn(*args)
            else:
                fn(*args, pj[:, 0:512])
            done.add(key)

        def ensure(key):
            # PE is in-order: an instruction whose operand-producing unit is
            # emitted later deadlocks the queue.  Force-pop until produced.
            while key not in done:
                assert fill, f"dependency {key} not in fill queue"
                pop_fill(0)

        def pop_pv():
            s_, m_, qh_, j_, pts = pv_q.popleft()
            ensure(("v", j_))
            pv_step(m_, qh_, j_, pts)
            if j_ == SB - 1:
                finalize(m_, qh_, last=(m_, qh_) == (1, 1))
                if (m_, qh_) == (1, 0):
                    for sbi in range(8):
                        for nb in range(2):
                            fill.append((("op", sbi, nb), oproj_mid, (sbi, nb)))


        def emit_exp(pt_ap, sc_ap, width):
            nc.scalar.activation(pt_ap, sc_ap, EXP)
            act_ns[0] += width * 0.8333 + 185.0

        # prologue: the first N_PRO steps run as two 512-wide passes — all
        # qq0 scores+exps first (they only need wq + x qs 0:512), then the
        # qq1 pass once the x qs 512:1024 DMA has landed.  This starts the
        # ACT stream ~8us earlier than a full-width first step.
        pro_pts = []
        for s in range(N_PRO):
            m, qh, j = steps[s]
            pts = []
            for r in range(2):
                pt = p_pool.tile([128, 1024], BF16,
                                 tag=f"p{pcnt[0] % NP}", name="pt")
                pcnt[0] += 1
                pts.append(pt)
            pro_pts.append(pts)
        for qq in range(2):
            for c in range(qq * 512, (qq + 1) * 512, 256):
                ensure(("qT", 0, c))
            for s in range(N_PRO):
                m, qh, j = steps[s]
                ensure(("kT", m, (j * 128 // 256) * 256))
                for r in range(2):
                    nc.tensor.matmul(
                        sc[r][:, qq * 512:(qq + 1) * 512],
                        kT[m][r * 64:r * 64 + 64, j * 128:(j + 1) * 128],
                        qT[m][r * 64:r * 64 + 64, qq * 512:(qq + 1) * 512],
                        start=True,
                        stop=True,
                    )
                    rows[0] += 512
                    emit_exp(pro_pts[s][r][:, qq * 512:(qq + 1) * 512],
                             sc[r][:, qq * 512:(qq + 1) * 512], 512)
                if qq == 1 and fill:
                    # weave projection pops between the second-pass scores
                    # so the step-4 scores aren't stuck behind them all
                    pop_fill(s)
                    if fill:
                        pop_fill(s)

        for s, (m, qh, j) in enumerate(steps):
            ensure(("kT", m, (j * 128 // 256) * 256))
            if s < N_PRO:
                pts = pro_pts[s]   # scores+exps already emitted above
            else:
                pts = []
                for c in range(qh * 1024, (qh + 1) * 1024, 256):
                    ensure(("qT", m, c))
                for r in range(2):
                    for qq in range(2):
                        nc.tensor.matmul(
                            sc[r][:, qq * 512:(qq + 1) * 512],
                            kT[m][r * 64:r * 64 + 64, j * 128:(j + 1) * 128],
                            qT[m][r * 64:r * 64 + 64,
                                  qh * 1024 + qq * 512:qh * 1024 + (qq + 1) * 512],
                            start=True,
                            stop=True,
                        )
                    rows[0] += 1024
                    pt = p_pool.tile([128, 1024], BF16,
                                     tag=f"p{pcnt[0] % NP}", name="pt")
                    pcnt[0] += 1
                    emit_exp(pt[:], sc[r][:], 1024)
                    pts.append(pt)
            pv_q.append((s, m, qh, j, pts))

            # pacing: emit PE work to track the ACT stream (rows whose PE
            # time matches cumulative ACT time), never popping a PV before
            # its exps have had a step to run (and not before wv landed).
            target = act_ns[0] * 2.25
            nfill = 0
            while pv_q or fill:
                can_pv = pv_q and pv_q[0][0] < s and s >= 6
                if can_pv and len(pv_q) > 16:
                    pop_pv()
                    continue
                if rows[0] >= target:
                    break
                if fill and nfill < 2:
                    pop_fill(s)
                    nfill += 1
                elif can_pv:
                    pop_pv()
                else:
                    break

        while pv_q:
            pop_pv()
        while fill:
            pop_fill(63)

        assert not fill
    return nc


_NC_CACHE = None


def _get_nc():
    global _NC_CACHE
    if _NC_CACHE is None:
        _NC_CACHE = build_nc()
    return _NC_CACHE


_EXEC_CACHE = None


def _get_executor():
    """Build + jit the SPMD executable once; reuse across kernel() calls.

    Mirrors concourse.bass2jax.run_bass_via_pjrt, which re-jits on every
    call (full retrace + executable reload); caching shaves seconds/call."""
    global _EXEC_CACHE
    if _EXEC_CACHE is not None:
        return _EXEC_CACHE
    import jax
    from jax.sharding import Mesh, PartitionSpec
    from jax.experimental.shard_map import shard_map
    from concourse import bass2jax as b2j

    nc = _get_nc()
    b2j.install_neuronx_cc_hook()
    assert nc.dbg_addr is None
    partition_name = (
        nc.partition_id_tensor.name if nc.partition_id_tensor is not None else None
    )

    in_names, out_names, out_avals = [], [], []
    for alloc in nc.m.functions[0].allocations:
        if not isinstance(alloc, mybir.MemoryLocationSet):
            continue
        name = alloc.memorylocations[0].name
        if alloc.kind == "ExternalInput":
            if name != partition_name:
                in_names.append(name)
        elif alloc.kind == "ExternalOutput":
            out_names.append(name)
            out_avals.append(
                jax.core.ShapedArray(
                    tuple(alloc.tensor_shape), mybir.dt.np(alloc.dtype)
                )
            )
    n_params = len(in_names)
    n_outs = len(out_avals)
    all_names = in_names + out_names
    if partition_name is not None:
        all_names = all_names + [partition_name]

    def _body(*args):
        operands = list(args)
        if partition_name is not None:
            operands.append(b2j.partition_id_tensor())
        outs = b2j._bass_exec_p.bind(
            *operands,
            out_avals=tuple(out_avals),
            in_names=tuple(all_names),
            out_names=tuple(out_names),
            lowering_input_output_aliases=(),
            sim_require_finite=True,
            sim_require_nnan=True,
            nc=nc,
        )
        return tuple(outs)

    devices = jax.devices()[:NCORES]
    mesh = Mesh(np.asarray(devices), ("core",))
    donate = tuple(range(n_params, n_params + n_outs))
    sharded = jax.jit(
        shard_map(
            _body,
            mesh=mesh,
            in_specs=(PartitionSpec("core"),) * (n_params + n_outs),
            out_specs=(PartitionSpec("core"),) * n_outs,
            check_rep=False,
        ),
        donate_argnums=donate,
        keep_unused=True,
    )
    import jax.numpy as jnp

    zero_shardings = [
        jax.sharding.NamedSharding(mesh, PartitionSpec("core"))
    ] * n_outs

    @jax.jit
    def _make_zeros():
        return tuple(
            jax.lax.with_sharding_constraint(
                jnp.zeros((NCORES * a.shape[0], *a.shape[1:]), a.dtype), sh
            )
            for a, sh in zip(out_avals, zero_shardings)
        )

    _EXEC_CACHE = {
        "sharded": sharded,
        "make_zeros": _make_zeros,
        "in_names": in_names,
        "out_names": out_names,
        "out_avals": out_avals,
    }
    return _EXEC_CACHE


def _run_spmd(in_maps):
    ex = _get_executor()
    concat_in = [
        np.concatenate([np.asarray(m[name]) for m in in_maps], axis=0)
        for name in ex["in_names"]
    ]
    concat_zeros = ex["make_zeros"]()
    out_arrs = ex["sharded"](*concat_in, *concat_zeros)
    results = []
    for c in range(NCORES):
        results.append({
            name: np.asarray(out_arrs[i]).reshape(
                NCORES, *ex["out_avals"][i].shape
            )[c]
            for i, name in enumerate(ex["out_names"])
        })
    return results


def _shard_inputs(x, Wq, Wk, Wv, Wo):
    import ml_dtypes

    scale = np.float32(1.0 / np.sqrt(HD))
    in_maps = []
    xT_b = [np.ascontiguousarray(x[b].T).astype(ml_dtypes.bfloat16) for b in range(B)]
    for c in range(NCORES):
        b, g = divmod(c, GROUPS)
        sl = slice(g * E, (g + 1) * E)
        in_maps.append({
            "xT": xT_b[b],
            "wqT": np.ascontiguousarray(Wq[sl, :].T * scale).astype(ml_dtypes.bfloat16),
            "wkT": np.ascontiguousarray(Wk[sl, :].T).astype(ml_dtypes.bfloat16),
            "wvT": np.ascontiguousarray(Wv[sl, :].T).astype(ml_dtypes.bfloat16),
            "woT": np.ascontiguousarray(Wo[:, sl].T).astype(ml_dtypes.bfloat16),
        })
    return in_maps


_FAST_PATH_OK = True


def kernel(x, Wq, Wk, Wv, Wo, bo):
    global _FAST_PATH_OK
    x = np.asarray(x, dtype=np.float32)
    in_maps = _shard_inputs(
        x,
        np.asarray(Wq, dtype=np.float32),
        np.asarray(Wk, dtype=np.float32),
        np.asarray(Wv, dtype=np.float32),
        np.asarray(Wo, dtype=np.float32),
    )
    results = None
    if _FAST_PATH_OK:
        try:
            results = _run_spmd(in_maps)
        except Exception:
            _FAST_PATH_OK = False
    if results is None:
        # portable fallback: stock SPMD runner (handles native-device
        # environments and anything the cached-PJRT fast path can't)
        results = run_bass_kernel_spmd(
            _get_nc(), in_maps, list(range(NCORES))
        ).results
    bo = np.asarray(bo, dtype=np.float32)
    out = np.empty((B, S, D), dtype=np.float32)
    for b in range(B):
        acc = np.zeros((S, D), dtype=np.float64)
        for g in range(GROUPS):
            acc += results[b * GROUPS + g]["out_partial"].astype(np.float64)
        out[b] = (acc + bo.astype(np.float64)).astype(np.float32)
    return out


# revision 78
# speedup vs baseline: 1.0583x; 1.0025x over previous
"""Multi-head attention (B=2, S=2048, D=1024, H=16) on 8 Trainium2 NeuronCores.

Sharding: core c handles batch b = c//4 and head group g = c%4 (4 heads, 256
model dims).  Each core computes q/k/v projections for its heads, attention,
and a partial output projection (row-parallel over its 256 head dims); the
host sums the 4 partials per batch and adds the bias.

Layouts / engine plan (v2 — PV reoriented, softmax normalize folded):
  All inputs bf16 (xT [d, s] host pre-transposed; wq pre-scaled by 1/sqrt(hd)).
  qT/kT [e(128 = head pair), s] bf16; scores computed transposed [ks, qs] in
  PSUM, exp on ACT -> p [ks, qs] bf16 in SBUF (ACT does nothing else until
  the drain, so the exp stream is the makespan spine at ~133us).
  PV is out[q(128), e(65)] with K=128 (lhsT = p block, rhs = v[ks, e+1] with a
  trailing ones column): the 65-col free dim runs at 1 cyc/row in bf16, and
  column 64 accumulates the softmax denominator per-q (a per-PARTITION
  scalar).  PSUM zeroing is lazy per 2KB zero region, so only the first
  write into each bank carries start_tensor_calc.  Normalization is folded
  into the PSUM evacuation: DVE reciprocal of the denominator column +
  tensor_scalar multiply -> ctxQ bf16 (GPSIMD cannot access PSUM on HW, so
  evacuations live on DVE mid-stream and DVE+ACT in the drain).
  ctxQ [q, e-pair] -> ctxT [e-pair, q] via the XBAR dma transpose (bf16,
  SBUF->SBUF), then the output projection contracts K=128 per head pair.
  Output partials staged f16 and DMA'd out; the host sums partials + bias.

PSUM (8 banks): sc0+sc1 [128,1024] f32 (4) + pva/pvb [128,7,65] + pvs
[128,2,65] (3, one pool each so they are bank-aligned) + pj [128,512] (1).

Emission is a 64-step pipeline (4 (m,qh) groups x 16 ks blocks): each step
emits the two score matmuls + exps, then pops PV steps and "filler" units
(qkv projection chunks, out-projection tiles) against a rows-emitted pacing
target so the PE queue tracks the ACT stream without starving it.
"""

import os
import sys

import numpy as np

for _p in ("/opt/trn_rl_repo", "/root/.axon_site/_ro/trn_rl_repo"):
    if os.path.isdir(_p) and _p not in sys.path:
        sys.path.insert(0, _p)

import bass_rust
import concourse.bass as bass
import concourse.mybir as mybir
import concourse.tile as tile
from concourse.bass_utils import run_bass_kernel_spmd
from concourse.vector_clock import ScopedClock, VectorClock
from contextlib import ExitStack
from collections import deque

F32 = mybir.dt.float32
F32R = mybir.dt.float32r
BF16 = mybir.dt.bfloat16
F16 = mybir.dt.float16
EXP = mybir.ActivationFunctionType.Exp

B = 2
S = 2048
D = 1024
H = 16
HD = 64
NCORES = 8
GROUPS = 4          # head groups (cores per batch)
HG = H // GROUPS    # heads per core = 4
E = HG * HD         # head dims per core = 256
KT = D // 128       # contraction tiles over model dim = 8
SB = S // 128       # 128-row s blocks = 16
NP = 40             # p-tile ring size (bf16 [128,1024] tiles)

_carrier_counter = [0]


def _split_multi_waits(ordered):
    """This walrus build allows one sync wait per instruction; Tile's wait
    assignment can attach several.  Hoist extras onto same-engine InstNoOp
    carriers placed immediately before the instruction."""
    for bb_name, insts in ordered.items():
        new_list = []
        for inst in insts:
            si = inst.sync_info
            waits = list(si.on_wait) if si is not None else []
            if len(waits) > 1:
                for w in waits[:-1]:
                    _carrier_counter[0] += 1
                    carrier = mybir.InstNoOp(
                        name=f"I-waitc-{_carrier_counter[0]}", ins=[], outs=[]
                    )
                    carrier.engine = inst.engine
                    carrier.sync_info = bass_rust.SyncInfo(on_wait=[w], on_update=[])
                    new_list.append(carrier)
                inst.sync_info = bass_rust.SyncInfo(
                    on_wait=[waits[-1]],
                    on_update=list(si.on_update) if si is not None else [],
                )
            new_list.append(inst)
        ordered[bb_name] = new_list


class _TileContext(tile.TileContext):
    """TileContext adapted to the one-sync-wait-per-instruction walrus."""

    def _lower_ordered_insts(self, ordered):
        _split_multi_waits(ordered)
        return super()._lower_ordered_insts(ordered)

    def _drain_and_barrier(self, tick_clock, wait_clock):
        gc = tick_clock.global_clock
        for proc in range(len(gc)):
            if gc[proc] <= 0:
                continue
            cur = VectorClock([0 if i == proc else gc[i] for i in range(len(gc))])
            nop = self.nc.sync.nop()
            wait_clock.add_sem_waits(
                nop.ins, ScopedClock({None: gc}), ScopedClock({None: cur})
            )
        drain_inst = self.nc.sync.drain()
        wait_clock.add_sem_waits(
            drain_inst.ins, ScopedClock({None: gc}), ScopedClock({None: gc.copy()})
        )
        self.nc.all_engine_barrier()
        assert self.sems is not None
        popped = self.nc._tile_sem_poison_stack.pop()
        assert popped is self._sem_poison
        self.nc.clear_and_free_semaphores(list(self.sems.allocated().values()))
        self.nc.all_engine_barrier()


def build_nc():
    nc = bass.Bass()
    xT = nc.declare_dram_parameter("xT", [D, S], BF16, isOutput=False)
    wqT = nc.declare_dram_parameter("wqT", [D, E], BF16, isOutput=False)
    wkT = nc.declare_dram_parameter("wkT", [D, E], BF16, isOutput=False)
    wvT = nc.declare_dram_parameter("wvT", [D, E], BF16, isOutput=False)
    woT = nc.declare_dram_parameter("woT", [E, D], BF16, isOutput=False)
    out = nc.declare_dram_parameter("out_partial", [S, D], F16, isOutput=True)

    with _TileContext(nc) as tc, ExitStack() as ctx:
        sb = ctx.enter_context(tc.tile_pool(name="sb", bufs=1))
        x_sb = sb.tile([128, KT, S], BF16, tag="x", name="x_sb")
        wq_sb = sb.tile([128, KT, E], BF16, tag="wq", name="wq_sb")
        wk_sb = sb.tile([128, KT, E], BF16, tag="wk", name="wk_sb")
        wv_sb = sb.tile([128, KT, E], BF16, tag="wv", name="wv_sb")
        wo_sb = sb.tile([128, 2, D], BF16, tag="wo", name="wo_sb")
        qT = [sb.tile([128, S], BF16, tag=f"qT{m}", name=f"qT{m}") for m in range(2)]
        kT = [sb.tile([128, S], BF16, tag=f"kT{m}", name=f"kT{m}") for m in range(2)]
        v_sb = sb.tile([128, SB, HG, HD + 1], BF16, tag="v", name="v_sb")
        ctxQ = [sb.tile([128, SB, 128], BF16, tag=f"cq{m}", name=f"cq{m}")
                for m in range(2)]
        ctxT = [sb.tile([128, S], BF16, tag=f"ct{m}", name=f"ct{m}")
                for m in range(2)]
        rec = [sb.tile([128, 2, SB], F32, tag=f"rec{m}", name=f"rec{m}")
               for m in range(2)]

        p_pool = ctx.enter_context(tc.tile_pool(name="pp", bufs=1))
        st_pool = ctx.enter_context(tc.tile_pool(name="st", bufs=1))

        ps_sc = ctx.enter_context(tc.tile_pool(name="ps_sc", bufs=1, space="PSUM"))
        ps_pva = ctx.enter_context(tc.tile_pool(name="ps_pva", bufs=1, space="PSUM"))
        ps_pvb = ctx.enter_context(tc.tile_pool(name="ps_pvb", bufs=1, space="PSUM"))
        ps_pvs = ctx.enter_context(tc.tile_pool(name="ps_pvs", bufs=1, space="PSUM"))
        ps_pj = ctx.enter_context(tc.tile_pool(name="ps_pj", bufs=1, space="PSUM"))

        sc = [ps_sc.tile([128, 1024], F32, tag=f"sc{r}", name=f"sc{r}")
              for r in range(2)]
        pva = ps_pva.tile([128, 7, HD + 1], F32, tag="pva", name="pva")
        pvb = ps_pvb.tile([128, 7, HD + 1], F32, tag="pvb", name="pvb")
        pvs = ps_pvs.tile([128, 2, HD + 1], F32, tag="pvs", name="pvs")
        pj = ps_pj.tile([128, 512], F32, tag="pj", name="pj")

        # ---- input DMAs ----
        # Priority-interleaved so the phase-A projections start after the
        # first (wq_k, x_k) pair instead of after the whole weight load:
        # sync queue carries the critical path (wq/wk + x for qs 0:1024),
        # gpsimd the rest (wv, wo, x tail).
        # preload the Exp activation table while DMAs stream (saves the
        # 1.3us implicit table load before the first real exp)
        warm = sb.tile([1, 512], BF16, tag="warm", name="warm")
        warmf = sb.tile([1, 8], F32, tag="warmf", name="warmf")
        nc.vector.memset(warm[0:1, :], 0.0)
        nc.vector.memset(warmf[0:1, 0:8], 0.0)
        nc.scalar.activation(warmf[0:1, 0:8], warmf[0:1, 0:8], EXP)
        # PE p-state warmup: a dep-free dummy matmul train ramps the tensor
        # engine to full clock before the first projection lands (~6us in)
        for _ in range(14):
            nc.tensor.matmul(pj[0:1, 0:512], warm[0:1, 0:1],
                             warm[0:1, 0:512], start=True, stop=True)

        # Batched loads (one DMA per tensor/chunk: per-DMA queue overhead
        # ~0.6us makes many small DMAs startup-dominant).  Critical stream
        # on sync: wq, x qs 0:512 (unblocks the qq-split prologue), wk,
        # x qs 512:1024, wv, x qs 1024:1536.
        def w_load(eng, dst, src):
            eng.dma_start(
                dst[:, :, :], src[:, :].rearrange("(k p) e -> p k e", p=128)
            )

        def x_load(eng, nb):
            eng.dma_start(
                x_sb[:, :, nb * 512:(nb + 1) * 512],
                xT[:, nb * 512:(nb + 1) * 512].rearrange(
                    "(k p) s -> p k s", p=128
                ),
            )

        def w_half(eng, dst, srcT, m):
            eng.dma_start(
                dst[:, :, m * 128:(m + 1) * 128],
                srcT[:, m * 128:(m + 1) * 128].rearrange(
                    "(k p) e -> p k e", p=128
                ),
            )

        w_half(nc.sync, wq_sb, wqT, 0)
        x_load(nc.sync, 0)
        w_half(nc.sync, wk_sb, wkT, 0)
        x_load(nc.sync, 1)
        w_load(nc.sync, wv_sb, wvT)
        w_half(nc.sync, wq_sb, wqT, 1)
        w_half(nc.sync, wk_sb, wkT, 1)
        x_load(nc.sync, 2)
        # ones column of v (softmax denominator rides the PV matmul)
        nc.gpsimd.memset(v_sb[:, :, :, HD:HD + 1], 1.0)

        # ---- emission helpers ----
        rows = [0]          # PE rows emitted so far (cost-model pacing)

        # rotating psum slots for projection/out-proj work.  pj is the
        # steady-state slot; during phase A and the tail the (then idle)
        # score tiles provide 4 more bank-aligned [128,512] slots.
        def slot_views():
            return [pj[:, 0:512], sc[0][:, 0:512], sc[0][:, 512:1024],
                    sc[1][:, 0:512], sc[1][:, 512:1024]]

        def qk_half(w_sb, dst, m, c0, slot, width=256):
            # one column block of the q or k projection for pair m
            for k in range(KT):
                nc.tensor.matmul(
                    slot[:, 0:width],
                    w_sb[:, k, m * 128:(m + 1) * 128],
                    x_sb[:, k, c0:c0 + width],
                    start=(k == 0),
                    stop=(k == KT - 1),
                )
            with nc.allow_low_precision("q/k rounded to bf16 for scores"):
                nc.vector.tensor_copy(dst[:, c0:c0 + width], slot[:, 0:width])
            rows[0] += KT * width

        def v_chunk(sbi, slot):
            for k in range(KT):
                nc.tensor.matmul(
                    slot[:, 0:256],
                    x_sb[:, k, sbi * 128:(sbi + 1) * 128],
                    wv_sb[:, k, :],
                    start=(k == 0),
                    stop=(k == KT - 1),
                )
            with nc.allow_low_precision("v rounded to bf16 for the PV matmul"):
                nc.vector.tensor_copy(
                    v_sb[:, sbi, :, 0:HD],
                    slot[:, 0:256].rearrange("p (h e) -> p h e", h=HG),
                )
            rows[0] += KT * 256

        st_cnt = [0]

        st_mid = {}

        def oproj_mid(sbi, nb):
            # one d-half of an s block of the qh0 out-projection (mid-stream,
            # single pj slot; per-nb units so the slot WAR sits between pops)
            if sbi not in st_mid:
                st_mid[sbi] = st_pool.tile([128, 1024], F16,
                                           tag=f"st{sbi % 4}", name="st")
            st = st_mid[sbi]
            for m in range(2):
                nc.tensor.matmul(
                    pj[:, 0:512],
                    ctxT[m][:, sbi * 128:(sbi + 1) * 128],
                    wo_sb[:, m, nb * 512:(nb + 1) * 512],
                    start=(m == 0),
                    stop=(m == 1),
                )
            with nc.allow_low_precision("output partial staged as f16"):
                nc.vector.tensor_copy(st[:, nb * 512:(nb + 1) * 512],
                                      pj[:, 0:512])
            if nb == 1:
                eng = nc.sync if sbi % 2 == 0 else nc.gpsimd
                eng.dma_start(out[sbi * 128:(sbi + 1) * 128, :], st[:])
            rows[0] += 2 * 512

        def oproj(sbi, slotA=None, slotB=None, tail=False):
            # both d-halves of one s block; staged f16 and stored with a
            # single DMA (per-DMA queue cost ~0.5us makes 32 stores pricey)
            i = st_cnt[0]
            st_cnt[0] += 1
            slotA = pj[:, 0:512] if slotA is None else slotA
            slotB = pj[:, 0:512] if slotB is None else slotB
            st = st_pool.tile([128, 1024], F16, tag=f"st{i % 4}", name="st")
            for nb, slot in ((0, slotA), (1, slotB)):
                for m in range(2):
                    nc.tensor.matmul(
                        slot[:, 0:512],
                        ctxT[m][:, sbi * 128:(sbi + 1) * 128],
                        wo_sb[:, m, nb * 512:(nb + 1) * 512],
                        start=(m == 0),
                        stop=(m == 1),
                    )
                with nc.allow_low_precision("output partial staged as f16"):
                    dst = st[:, nb * 512:(nb + 1) * 512]
                    if (i + nb) % 2 == 0:
                        nc.vector.tensor_copy(dst, slot[:, 0:512])
                    else:
                        nc.scalar.copy(dst, slot[:, 0:512])
            dma_eng = (nc.sync, nc.gpsimd, nc.scalar, nc.sync,
                       nc.gpsimd, nc.scalar, nc.sync, nc.scalar)[(i - 8) % 8]
            dma_eng.dma_start(out[sbi * 128:(sbi + 1) * 128, :], st[:])
            rows[0] += 4 * 512

        def pv_step(m, qh, j, pts):
            # PSUM zeroing is lazy per 2KB zero region: start=True marks the
            # whole bank pending-zero, and each sub-tile's first touch then
            # overwrites while later touches accumulate.  So only the FIRST
            # write into each bank carries start, the LAST carries stop.
            for r in range(2):
                for qb in range(8):
                    if qb < 7:
                        dst = (pva if r == 0 else pvb)[:, qb, :]
                        first = j == 0 and qb == 0
                        stop = j == SB - 1 and qb == 6
                    else:
                        dst = pvs[:, r, :]
                        first = j == 0 and r == 0
                        stop = j == SB - 1 and r == 1
                    nc.tensor.matmul(
                        dst,
                        pts[r][:, qb * 128:(qb + 1) * 128],
                        v_sb[:, j, 2 * m + r, :],
                        start=first,
                        stop=stop,
                        skip_group_check=True,
                    )
            rows[0] += 16 * (HD + 1)

        def finalize(m, qh, last=False):
            # reciprocal of the denominator columns, then normalize-fold the
            # PSUM evacuation into ctxQ (bf16), then transpose into ctxT.
            # Streamed per q block (fold r0 on DVE, r1 on Pool, transpose
            # immediately after) so the tail drains as a pipeline.
            for r in range(2):
                pv = pva if r == 0 else pvb
                nc.vector.reciprocal(
                    rec[m][:, r, qh * 8:qh * 8 + 7],
                    pv[:, 0:7, HD:HD + 1].rearrange("p a b -> p (a b)"),
                )
                nc.vector.reciprocal(
                    rec[m][:, r, qh * 8 + 7:qh * 8 + 8],
                    pvs[:, r, HD:HD + 1],
                )
            with nc.allow_low_precision("attention context rounded to bf16"):
                def fold(qb, r):
                    src = (pva if r == 0 else pvb)[:, qb, 0:HD] if qb < 7 \
                        else pvs[:, r, 0:HD]
                    dst = ctxQ[m][:, qh * 8 + qb, r * 64:(r + 1) * 64]
                    sca = rec[m][:, r, qh * 8 + qb:qh * 8 + qb + 1]
                    if last and r == 1:
                        # ACT is free after the last exp; its Copy-with-
                        # scale IS the normalize-fold (GPSIMD cannot touch
                        # PSUM on hardware, so only DVE/ACT can)
                        nc.scalar.activation(
                            dst, src,
                            mybir.ActivationFunctionType.Copy,
                            scale=sca,
                        )
                    else:
                        nc.vector.tensor_scalar_mul(dst, src, sca)

                if not last:
                    for qb in range(8):
                        fold(qb, 0)
                        fold(qb, 1)
                        qg = qh * 8 + qb
                        # transpose via the DMA XBAR (latency hides behind
                        # the still-running exp stream)
                        nc.sync.dma_start_transpose(
                            ctxT[m][:, qg * 128:(qg + 1) * 128],
                            ctxQ[m][:, qg, :],
                        )
                else:
                    # tail drain, stage-major: folds (DVE r0 / ACT r1),
                    # XBAR transposes (write ctxT directly; their latency
                    # pipelines under the folds), then the out-projection
                    # stream over 5 PSUM slots
                    for qb in range(8):
                        fold(qb, 0)
                        fold(qb, 1)
                        qg = qh * 8 + qb
                        nc.sync.dma_start_transpose(
                            ctxT[m][:, qg * 128:(qg + 1) * 128],
                            ctxQ[m][:, qg, :],
                        )
                    slots5 = [pj[:, 0:512], sc[0][:, 0:512],
                              sc[0][:, 512:1024], sc[1][:, 0:512],
                              sc[1][:, 512:1024]]
                    for qb in range(8):
                        oproj(qh * 8 + qb, slots5[(2 * qb) % 5],
                              slots5[(2 * qb + 1) % 5], tail=True)

        # ---- phase A: minimum needed for the first (qq-split) scores ----
        done = set()    # completed filler units, keyed for dep-forced pops
        slots = slot_views()
        phase_a = (
            [(("qT", 0, 0), (wq_sb, qT[0], 0, 0, 256)),
             (("qT", 0, 256), (wq_sb, qT[0], 0, 256, 256)),
             # kT 0:256 as two 128-wide minis: the first scores only need
             # cols 0:128, so the exp stream starts one mini earlier
             (None, (wk_sb, kT[0], 0, 0, 128)),
             (("kT", 0, 0), (wk_sb, kT[0], 0, 128, 128))]
        )
        for i, (key, args) in enumerate(phase_a):
            qk_half(args[0], args[1], args[2], args[3], slots[i % 5],
                    width=args[4])
            if key is not None:
                done.add(key)

        # late DMA stream (gpsimd): gated behind the first qT copy so it
        # doesn't dilute the critical sync-queue stream on the DMA engines.
        # DMA queue preps schedule by data deps, not engine order, so gate
        # each late DMA with a WAW dep: scribble into its destination from a
        # copy that depends on the first qT tile (ready ~9us in).
        with nc.allow_low_precision("gate scribbles, overwritten by DMAs"):
            nc.gpsimd.tensor_copy(x_sb[0:1, 0, 1536:1544], qT[0][0:1, 0:8])
            x_load(nc.gpsimd, 3)
            for m in range(2):
                nc.gpsimd.tensor_copy(wo_sb[0:1, m, 0:8], qT[0][0:1, 0:8])
                nc.gpsimd.dma_start(wo_sb[:, m, :], woT[m * 128:(m + 1) * 128, :])

        # ---- filler queue (deadline-ordered; pops may be dep-forced) ----
        fill = deque()

        def add_qk(w_sb, dst, m, c):
            t = "qT" if dst is qT[m] else "kT"
            fill.append(((t, m, c), qk_half, (w_sb, dst, m, c)))

        def add_v(j):
            fill.append((("v", j), v_chunk, (j,)))

        add_qk(wk_sb, kT[0], 0, 256)
        add_qk(wq_sb, qT[0], 0, 512)
        add_qk(wq_sb, qT[0], 0, 768)
        add_qk(wk_sb, kT[0], 0, 512)
        add_qk(wk_sb, kT[0], 0, 768)
        add_v(0)
        add_v(1)
        add_qk(wk_sb, kT[0], 0, 1024)
        add_qk(wq_sb, qT[0], 0, 1024)
        add_v(2)
        add_v(3)
        add_qk(wk_sb, kT[0], 0, 1280)
        add_qk(wq_sb, qT[0], 0, 1280)
        add_v(4)
        add_qk(wk_sb, kT[0], 0, 1536)
        add_qk(wq_sb, qT[0], 0, 1536)
        add_v(5)
        add_qk(wk_sb, kT[0], 0, 1792)
        add_qk(wq_sb, qT[0], 0, 1792)
        add_v(6)
        add_v(7)
        add_v(8)
        add_qk(wq_sb, qT[1], 1, 0)
        add_v(9)
        add_qk(wq_sb, qT[1], 1, 256)
        add_v(10)
        add_qk(wq_sb, qT[1], 1, 512)
        add_v(11)
        add_qk(wq_sb, qT[1], 1, 768)
        add_v(12)
        add_qk(wk_sb, kT[1], 1, 0)
        add_v(13)
        add_qk(wk_sb, kT[1], 1, 256)
        add_v(14)
        add_qk(wk_sb, kT[1], 1, 512)
        add_v(15)
        add_qk(wk_sb, kT[1], 1, 768)
        for c in range(1024, S, 256):
            add_qk(wq_sb, qT[1], 1, c)
            add_qk(wk_sb, kT[1], 1, c)

        # ---- main 64-step pipeline ----
        group_seq = [(0, 0), (0, 1), (1, 0), (1, 1)]
        steps = [(m, qh, j) for (m, qh) in group_seq for j in range(SB)]
        N_PRO = 4            # qq-split prologue steps (512-wide exps)

        pv_q = deque()
        pcnt = [0]
        act_ns = [0.0]       # cumulative ACT time emitted (pacing reference)
        PE_NS_PER_ROW = 1.0 / 2.4

        fill_cnt = [0]

        def pop_fill(s):
            key, fn, args = fill.popleft()
            if fn is oproj_mid:
                fn(*args)
            else:
                fn(*args, pj[:, 0:512])
            done.add(key)

        def ensure(key):
            # PE is in-order: an instruction whose operand-producing unit is
            # emitted later deadlocks the queue.  Force-pop until produced.
            while key not in done:
                assert fill, f"dependency {key} not in fill queue"
                pop_fill(0)

        def pop_pv():
            s_, m_, qh_, j_, pts = pv_q.popleft()
            ensure(("v", j_))
            pv_step(m_, qh_, j_, pts)
            if j_ == SB - 1:
                finalize(m_, qh_, last=(m_, qh_) == (1, 1))
                if (m_, qh_) == (1, 0):
                    for sbi in range(8):
                        for nb in range(2):
                            fill.append((("op", sbi, nb), oproj_mid, (sbi, nb)))


        def emit_exp(pt_ap, sc_ap, width):
            nc.scalar.activation(pt_ap, sc_ap, EXP)
            act_ns[0] += width * 0.8333 + 185.0

        # prologue: the first N_PRO steps run as two 512-wide passes — all
        # qq0 scores+exps first (they only need wq + x qs 0:512), then the
        # qq1 pass once the x qs 512:1024 DMA has landed.  This starts the
        # ACT stream ~8us earlier than a full-width first step.
        pro_pts = []
        for s in range(N_PRO):
            m, qh, j = steps[s]
            pts = []
            for r in range(2):
                pt = p_pool.tile([128, 1024], BF16,
                                 tag=f"p{pcnt[0] % NP}", name="pt")
                pcnt[0] += 1
                pts.append(pt)
            pro_pts.append(pts)
        for qq in range(2):
            for c in range(qq * 512, (qq + 1) * 512, 256):
                ensure(("qT", 0, c))
            for s in range(N_PRO):
                m, qh, j = steps[s]
                ensure(("kT", m, (j * 128 // 256) * 256))
                for r in range(2):
                    nc.tensor.matmul(
                        sc[r][:, qq * 512:(qq + 1) * 512],
                        kT[m][r * 64:r * 64 + 64, j * 128:(j + 1) * 128],
                        qT[m][r * 64:r * 64 + 64, qq * 512:(qq + 1) * 512],
                        start=True,
                        stop=True,
                    )
                    rows[0] += 512
                    emit_exp(pro_pts[s][r][:, qq * 512:(qq + 1) * 512],
                             sc[r][:, qq * 512:(qq + 1) * 512], 512)
                if qq == 1 and fill:
                    # weave projection pops between the second-pass scores
                    # so the step-4 scores aren't stuck behind them all
                    pop_fill(s)
                    if fill:
                        pop_fill(s)

        for s, (m, qh, j) in enumerate(steps):
            ensure(("kT", m, (j * 128 // 256) * 256))
            if s < N_PRO:
                pts = pro_pts[s]   # scores+exps already emitted above
            else:
                pts = []
                for c in range(qh * 1024, (qh + 1) * 1024, 256):
                    ensure(("qT", m, c))
                for r in range(2):
                    for qq in range(2):
                        nc.tensor.matmul(
                            sc[r][:, qq * 512:(qq + 1) * 512],
                            kT[m][r * 64:r * 64 + 64, j * 128:(j + 1) * 128],
                            qT[m][r * 64:r * 64 + 64,
                                  qh * 1024 + qq * 512:qh * 1024 + (qq + 1) * 512],
                            start=True,
                            stop=True,
                        )
                    rows[0] += 1024
                    pt = p_pool.tile([128, 1024], BF16,
                                     tag=f"p{pcnt[0] % NP}", name="pt")
                    pcnt[0] += 1
                    emit_exp(pt[:], sc[r][:], 1024)
                    pts.append(pt)
            pv_q.append((s, m, qh, j, pts))

            # pacing: emit PE work to track the ACT stream (rows whose PE
            # time matches cumulative ACT time), never popping a PV before
            # its exps have had a step to run (and not before wv landed).
            target = act_ns[0] * 2.25
            nfill = 0
            while pv_q or fill:
                can_pv = pv_q and pv_q[0][0] < s and s >= 6
                if can_pv and len(pv_q) > 18:
                    pop_pv()
                    continue
                if rows[0] >= target:
                    break
                if fill and nfill < 2:
                    pop_fill(s)
                    nfill += 1
                elif can_pv:
                    pop_pv()
                else:
                    break

        while pv_q:
            pop_pv()
        while fill:
            pop_fill(63)

        assert not fill
    return nc


_NC_CACHE = None


def _get_nc():
    global _NC_CACHE
    if _NC_CACHE is None:
        _NC_CACHE = build_nc()
    return _NC_CACHE


_EXEC_CACHE = None


def _get_executor():
    """Build + jit the SPMD executable once; reuse across kernel() calls.

    Mirrors concourse.bass2jax.run_bass_via_pjrt, which re-jits on every
    call (full retrace + executable reload); caching shaves seconds/call."""
    global _EXEC_CACHE
    if _EXEC_CACHE is not None:
        return _EXEC_CACHE
    import jax
    from jax.sharding import Mesh, PartitionSpec
    from jax.experimental.shard_map import shard_map
    from concourse import bass2jax as b2j

    nc = _get_nc()
    b2j.install_neuronx_cc_hook()
    assert nc.dbg_addr is None
    partition_name = (
        nc.partition_id_tensor.name if nc.partition_id_tensor is not None else None
    )

    in_names, out_names, out_avals = [], [], []
    for alloc in nc.m.functions[0].allocations:
        if not isinstance(alloc, mybir.MemoryLocationSet):
            continue
        name = alloc.memorylocations[0].name
        if alloc.kind == "ExternalInput":
            if name != partition_name:
                in_names.append(name)
        elif alloc.kind == "ExternalOutput":
            out_names.append(name)
            out_avals.append(
                jax.core.ShapedArray(
                    tuple(alloc.tensor_shape), mybir.dt.np(alloc.dtype)
                )
            )
    n_params = len(in_names)
    n_outs = len(out_avals)
    all_names = in_names + out_names
    if partition_name is not None:
        all_names = all_names + [partition_name]

    def _body(*args):
        operands = list(args)
        if partition_name is not None:
            operands.append(b2j.partition_id_tensor())
        outs = b2j._bass_exec_p.bind(
            *operands,
            out_avals=tuple(out_avals),
            in_names=tuple(all_names),
            out_names=tuple(out_names),
            lowering_input_output_aliases=(),
            sim_require_finite=True,
            sim_require_nnan=True,
            nc=nc,
        )
        return tuple(outs)

    devices = jax.devices()[:NCORES]
    mesh = Mesh(np.asarray(devices), ("core",))
    donate = tuple(range(n_params, n_params + n_outs))
    sharded = jax.jit(
        shard_map(
            _body,
            mesh=mesh,
            in_specs=(PartitionSpec("core"),) * (n_params + n_outs),
            out_specs=(PartitionSpec("core"),) * n_outs,
            check_rep=False,
        ),
        donate_argnums=donate,
        keep_unused=True,
    )
    import jax.numpy as jnp

    zero_shardings = [
        jax.sharding.NamedSharding(mesh, PartitionSpec("core"))
    ] * n_outs

    @jax.jit
    def _make_zeros():
        return tuple(
            jax.lax.with_sharding_constraint(
                jnp.zeros((NCORES * a.shape[0], *a.shape[1:]), a.dtype), sh
            )
            for a, sh in zip(out_avals, zero_shardings)
        )

    _EXEC_CACHE = {
        "sharded": sharded,
        "make_zeros": _make_zeros,
        "in_names": in_names,
        "out_names": out_names,
        "out_avals": out_avals,
    }
    return _EXEC_CACHE


def _run_spmd(in_maps):
    ex = _get_executor()
    concat_in = [
        np.concatenate([np.asarray(m[name]) for m in in_maps], axis=0)
        for name in ex["in_names"]
    ]
    concat_zeros = ex["make_zeros"]()
    out_arrs = ex["sharded"](*concat_in, *concat_zeros)
    results = []
    for c in range(NCORES):
        results.append({
            name: np.asarray(out_arrs[i]).reshape(
                NCORES, *ex["out_avals"][i].shape
            )[c]
            for i, name in enumerate(ex["out_names"])
        })
    return results


def _shard_inputs(x, Wq, Wk, Wv, Wo):
    import ml_dtypes

    scale = np.float32(1.0 / np.sqrt(HD))
    in_maps = []
    xT_b = [np.ascontiguousarray(x[b].T).astype(ml_dtypes.bfloat16) for b in range(B)]
    for c in range(NCORES):
        b, g = divmod(c, GROUPS)
        sl = slice(g * E, (g + 1) * E)
        in_maps.append({
            "xT": xT_b[b],
            "wqT": np.ascontiguousarray(Wq[sl, :].T * scale).astype(ml_dtypes.bfloat16),
            "wkT": np.ascontiguousarray(Wk[sl, :].T).astype(ml_dtypes.bfloat16),
            "wvT": np.ascontiguousarray(Wv[sl, :].T).astype(ml_dtypes.bfloat16),
            "woT": np.ascontiguousarray(Wo[:, sl].T).astype(ml_dtypes.bfloat16),
        })
    return in_maps


_FAST_PATH_OK = True


def kernel(x, Wq, Wk, Wv, Wo, bo):
    global _FAST_PATH_OK
    x = np.asarray(x, dtype=np.float32)
    in_maps = _shard_inputs(
        x,
        np.asarray(Wq, dtype=np.float32),
        np.asarray(Wk, dtype=np.float32),
        np.asarray(Wv, dtype=np.float32),
        np.asarray(Wo, dtype=np.float32),
    )
    results = None
    if _FAST_PATH_OK:
        try:
            results = _run_spmd(in_maps)
        except Exception:
            _FAST_PATH_OK = False
    if results is None:
        # portable fallback: stock SPMD runner (handles native-device
        # environments and anything the cached-PJRT fast path can't)
        results = run_bass_kernel_spmd(
            _get_nc(), in_maps, list(range(NCORES))
        ).results
    bo = np.asarray(bo, dtype=np.float32)
    out = np.empty((B, S, D), dtype=np.float32)
    for b in range(B):
        acc = np.zeros((S, D), dtype=np.float64)
        for g in range(GROUPS):
            acc += results[b * GROUPS + g]["out_partial"].astype(np.float64)
        out[b] = (acc + bo.astype(np.float64)).astype(np.float32)
    return out
